# revision 5
# baseline (speedup 1.0000x reference)
"""Trainium2 Bass kernel for nn_MultiHeadSelfAttention_65025804862080 (v3).

Full inputs in, full output out. Core i handles batch b=i//4 and heads
{4j..4j+3} (j=i%4). The reference's no-transpose head split means head h's
Q/K/V derive from x tokens [128h, 128h+128) only, so QKV shards across
cores with zero duplication. RoPE is position-independent (reference
indexes cos/sin at the single position t=T) and folds into wq/wk on host.

v3 (vs v2): head-pair cycles exploit PE array packing. Token permutation
is mp-major: pi = mp*256 + w*128 + tt (m = 2mp+w = x column chunk of 64,
tt = token row in the head's 128-row x block), so V-projection drains are
identity copies and K/Q chunks are consumed in drain order. 64 pair-
cycles (4 units = pair x q-half, 16 k-slabs each):
  - S for the pair's two heads runs as concurrent 64x128 row-tiles
    (z0 rows 0:64, z1 rows 64:128): K=dh=64 no longer idles half the PE.
  - exp paces the loop (2 x [128,1024] per cycle); S leads its exp by
    half a cycle, so psS is exactly two [128,1024] tiles (one per z).
  - PV runs as 4 concurrent 128x32 col-tiles (z x d-chunk) accumulating
    both heads into ONE [128,1024] psum tile shaped as the outproj
    stationary; softmax row sums are 4 concurrent M=1 col-tiles into a
    single psum bank (ones-column matmuls riding the same pt streams).
  - normalization per unit: DVE reciprocal of the Z bank, K=1 broadcast
    matmuls at row/col tile positions, DVE multiply into ONP - no DRAM
    bounce, no cross-partition DMA.
  - remaining projections + output projection run as PE fillers inside
    the ACT-paced loop; only K0-5/Q0-3/V(l0,l1,cc0) are prologue.
"""

import sys

if "/opt/trn_rl_repo" not in sys.path:
    sys.path.insert(0, "/opt/trn_rl_repo")

from contextlib import ExitStack

import ml_dtypes
import numpy as np

import concourse.tile as tile
from concourse import bacc, mybir
from concourse.bass_utils import run_bass_kernel_spmd

B, T, DM, H, DH = 2, 2048, 1024, 16, 64
N_CORES = 8
HPC = 4          # heads per core
RB = 512         # x-row block per core
F32 = mybir.dt.float32
F32R = mybir.dt.float32r
BF16 = mybir.dt.bfloat16
EXPF = mybir.ActivationFunctionType.Exp
ADD = mybir.AluOpType.add

PRO_Q = 4        # Q m2 chunks in prologue
PRO_K = 6        # K m2 chunks in prologue
NG = 64          # pair-cycles
UNITS = [(0, 0), (1, 0), (0, 1), (1, 1)]  # (pair a, q-half hf)


def build_program():
    nc = bacc.Bacc(
        "TRN2", target_bir_lowering=False, debug=False, num_devices=N_CORES
    )

    xTp = nc.dram_tensor("xTp", [128, 8 * RB], BF16, kind="ExternalInput").ap()
    wqT = nc.dram_tensor("wqT", [8, 128, DM], BF16, kind="ExternalInput").ap()
    wkT = nc.dram_tensor("wkT", [8, 128, DM], BF16, kind="ExternalInput").ap()
    wvT = nc.dram_tensor("wvT", [8, 128, DM], BF16, kind="ExternalInput").ap()
    wop = nc.dram_tensor("wop", [2, 128, DM], BF16, kind="ExternalInput").ap()
    bqp = nc.dram_tensor("bqp", [128, 8], F32, kind="ExternalInput").ap()
    bkp = nc.dram_tensor("bkp", [128, 8], F32, kind="ExternalInput").ap()
    bvr = nc.dram_tensor("bvr", [1, DM], F32R, kind="ExternalInput").ap()
    ones1 = nc.dram_tensor("ones1", [1, 128], F32R, kind="ExternalInput").ap()
    out = nc.dram_tensor("out", [T, DM], BF16, kind="ExternalOutput").ap()

    with tile.TileContext(nc) as tc:
        _emit(nc, tc, xTp, wqT, wkT, wvT, wop, bqp, bkp, bvr, ones1, out)

    nc.compile()
    return nc


def _emit(nc, tc, xTp, wqT, wkT, wvT, wop, bqp, bkp, bvr, ones1, out):
    ctx = ExitStack()
    with ctx:
        singles = ctx.enter_context(tc.tile_pool(name="singles", bufs=1))
        big = ctx.enter_context(tc.tile_pool(name="big", bufs=1))
        w_pool = ctx.enter_context(tc.tile_pool(name="wts", bufs=24))
        pt_pool = ctx.enter_context(tc.tile_pool(name="pt", bufs=5))
        stg_pool = ctx.enter_context(tc.tile_pool(name="stg", bufs=4))
        stgc_pool = ctx.enter_context(tc.tile_pool(name="stgc", bufs=4))

        # ---- big persistent tiles ----
        XT = big.tile([128, 8 * RB], BF16, tag="xt", name="XT")
        # QQ/KK: [z*64+dd, a*2048 + pi]; pi = mp*256 + w*128 + tt
        QQ = big.tile([128, 2 * T], BF16, tag="qq", name="QQ")
        KK = big.tile([128, 2 * T], BF16, tag="kk", name="KK")
        # VN[l]: [128(tt), 16*64]; slab s cols 64s:64s+64 (identity w/ psv)
        VN = [big.tile([128, 16 * 64], BF16, tag=f"vn{l}", name=f"vn{l}")
              for l in range(HPC)]
        # ONP[a]: normalized O^T: partition z*64+dd, free pi (outproj lhsT)
        ONP = [big.tile([128, T], BF16, tag=f"onp{a}", name=f"onp{a}")
               for a in range(2)]
        WOP = [big.tile([128, DM], BF16, tag=f"wop{a}", name=f"wopp{a}")
               for a in range(2)]
        recZ = big.tile([128, 512], BF16, tag="rcz", name="recZ")

        bq_sb = singles.tile([128, 8], F32, tag="bq", name="bq_sb")
        bk_sb = singles.tile([128, 8], F32, tag="bk", name="bk_sb")
        bv_sb = singles.tile([1, DM], F32R, tag="bv", name="bv_sb")
        ones_r = singles.tile([1, 128], F32R, tag="ones", name="ones_r")
        onescol = singles.tile([128, 1], BF16, tag="onec", name="onescol")
        onesb = singles.tile([128, 64], BF16, tag="onesb", name="onesb")
        nc.vector.memset(onescol, 1.0)
        nc.vector.memset(onesb, 1.0)

        # ---- initial DMAs ----
        x3o = XT.rearrange("p (k c) -> p k c", c=RB)
        x3i = xTp.rearrange("p (k c) -> p k c", c=RB)
        W = {}
        for nm in ("q", "k", "v"):
            for p in range(8):
                W[(nm, p)] = w_pool.tile([128, DM], BF16, tag="w",
                                         name=f"w{nm}{p}")
        for p in range(8):
            nsplit = 2 if p < 2 else 1
            for h in range(nsplit):
                w = 128 // nsplit
                nc.sync.dma_start(out=x3o[w * h:w * (h + 1), p, :],
                                  in_=x3i[w * h:w * (h + 1), p, :])
        for p in range(8):
            nsplit = 4 if p == 0 else 2
            for h in range(nsplit):
                w = 128 // nsplit
                nc.scalar.dma_start(
                    out=W[("q", p)][w * h:w * (h + 1), :],
                    in_=wqT[p, w * h:w * (h + 1), :])
            if p == 1:
                nc.scalar.dma_start(out=bq_sb, in_=bqp)
                nc.scalar.dma_start(out=bk_sb, in_=bkp)
                nc.scalar.dma_start(out=bv_sb, in_=bvr)
                nc.scalar.dma_start(out=ones_r, in_=ones1)
        for p in range(8):
            for h in range(2):
                nc.scalar.dma_start(out=W[("k", p)][64 * h:64 * (h + 1), :],
                                    in_=wkT[p, 64 * h:64 * (h + 1), :])
        for p in range(8):
            nc.scalar.dma_start(out=W[("v", p)], in_=wvT[p])

        # dest views for QK drains: col = a*2048 + m2*256 + w*128 + tt
        QQ5 = QQ.rearrange("p (a mp w tt) -> p a mp w tt", a=2, mp=8, w=2)
        KK5 = KK.rearrange("p (a mp w tt) -> p a mp w tt", a=2, mp=8, w=2)

        def qk_drain(ps, m2, b_sb, d5):
            """psq [128(w*64+dd), 512(a*256+zz*128+tt)] -> QQ/KK + bias."""
            srcs = [ps[0:64, :].rearrange("p (a zz t) -> p zz a t", a=2, zz=2),
                    ps[64:128, :].rearrange("p (a zz t) -> p zz a t",
                                            a=2, zz=2)]
            for w in range(2):  # same-partition pieces (zz == w)
                nc.vector.tensor_scalar(
                    out=d5[64 * w:64 * w + 64, :, m2, w, :],
                    in0=srcs[w][:, w, :, :],
                    scalar1=b_sb[64 * w:64 * w + 64, m2:m2 + 1],
                    scalar2=None, op0=ADD)
            stg = stg_pool.tile([128, 256], BF16, tag="stg", name=f"sg{m2}")
            st3 = stg.rearrange("p (a t) -> p a t", a=2)
            nc.vector.tensor_scalar(
                out=st3[0:64], in0=srcs[0][:, 1, :, :],
                scalar1=b_sb[0:64, m2:m2 + 1], scalar2=None, op0=ADD)
            nc.vector.tensor_scalar(
                out=st3[64:128], in0=srcs[1][:, 0, :, :],
                scalar1=b_sb[64:128, m2:m2 + 1], scalar2=None, op0=ADD)
            nc.sync.dma_start(out=d5[64:128, :, m2, 0, :], in_=st3[0:64])
            nc.sync.dma_start(out=d5[0:64, :, m2, 1, :], in_=st3[64:128])

        # ---------- prologue projections ----------
        with tc.tile_pool(name="psA", bufs=1, space="PSUM") as psA:
            psq = [psA.tile([128, RB], F32, tag=f"A{i}", name=f"psq{i}")
                   for i in range(8)]

            def proj_qk(nm, m2, b_sb, d5, slot):
                for p in range(8):
                    nc.tensor.matmul(
                        psq[slot][:],
                        W[(nm, p)][:, 128 * m2:128 * (m2 + 1)],
                        x3o[:, p, :], start=(p == 0), stop=(p == 7))
                qk_drain(psq[slot], m2, b_sb, d5)

            def proj_v(l, cc, slot):
                half = psq[slot][:, 0:512]
                for p in range(8):
                    nc.tensor.matmul(
                        half, x3o[:, p, 128 * l:128 * (l + 1)],
                        W[("v", p)][:, 512 * cc:512 * (cc + 1)],
                        start=(p == 0), stop=False)
                nc.tensor.matmul(
                    half, ones_r[0:1, 0:128],
                    bv_sb[0:1, 512 * cc:512 * (cc + 1)],
                    start=False, stop=True)
                nc.vector.tensor_copy(VN[l][:, 512 * cc:512 * (cc + 1)], half)

            for m2 in range(4):
                proj_qk("k", m2, bk_sb, KK5, m2)
                proj_qk("q", m2, bq_sb, QQ5, 4 + m2)
            for m2 in range(4, PRO_K):
                proj_qk("k", m2, bk_sb, KK5, m2)
            proj_v(0, 0, 0)
            proj_v(1, 0, 1)

        nc.sync.dma_start(out=WOP[0], in_=wop[0])
        nc.scalar.dma_start(out=WOP[1], in_=wop[1])

        # ---------- main loop ----------
        fillers = []

        with (
            tc.tile_pool(name="psS", bufs=1, space="PSUM") as psS,
            tc.tile_pool(name="psO", bufs=1, space="PSUM") as psO,
            tc.tile_pool(name="psZ", bufs=1, space="PSUM") as psZ,
            tc.tile_pool(name="psC", bufs=1, space="PSUM") as psC,
        ):
            AQ = QQ.rearrange("p (a q) -> p a q", a=2)
            AK = KK.rearrange("p (a q) -> p a q", a=2)

            def make_projqk_fillers(nm, m2, b_sb, d5):
                st = {}

                def mk(p):
                    def go():
                        if p == 0:
                            st["t"] = psC.tile([128, RB], F32, tag="c",
                                               name=f"f{nm}{m2}")
                        nc.tensor.matmul(
                            st["t"][:],
                            W[(nm, p)][:, 128 * m2:128 * (m2 + 1)],
                            x3o[:, p, :], start=(p == 0), stop=(p == 7))
                        if p == 7:
                            qk_drain(st["t"], m2, b_sb, d5)
                    return go
                for p in range(8):
                    fillers.append(mk(p))

            def make_projv_fillers(l, cc):
                st = {}

                def mk(p):
                    def go():
                        if p == 0:
                            st["t"] = psC.tile([128, 512], F32, tag="c",
                                               name=f"fv{l}{cc}")
                        nc.tensor.matmul(
                            st["t"][:], x3o[:, p, 128 * l:128 * (l + 1)],
                            W[("v", p)][:, 512 * cc:512 * (cc + 1)],
                            start=(p == 0), stop=False)
                        if p == 7:
                            nc.tensor.matmul(
                                st["t"][:], ones_r[0:1, 0:128],
                                bv_sb[0:1, 512 * cc:512 * (cc + 1)],
                                start=False, stop=True)
                            nc.vector.tensor_copy(
                                VN[l][:, 512 * cc:512 * (cc + 1)], st["t"][:])
                    return go
                for p in range(8):
                    fillers.append(mk(p))

            def make_outproj_fillers(sqs, use_act=False):
                def mk(sq, cc):
                    def go():
                        oc = psC.tile([128, 512], F32, tag="c",
                                      name=f"oc{sq}{cc}")
                        for a in range(2):
                            nc.tensor.matmul(
                                oc[:], ONP[a][:, 128 * sq:128 * sq + 128],
                                WOP[a][:, 512 * cc:512 * (cc + 1)],
                                start=(a == 0), stop=(a == 1))
                        stgc = stgc_pool.tile([128, 512], BF16, tag="sc",
                                              name=f"sc{sq}{cc}")
                        if use_act:
                            nc.scalar.activation(
                                stgc[:], oc[:],
                                mybir.ActivationFunctionType.Copy)
                        else:
                            nc.vector.tensor_copy(stgc[:], oc[:])
                        eng = nc.sync if (sq + cc) % 2 == 0 else nc.gpsimd
                        eng.dma_start(
                            out=out[128 * sq:128 * (sq + 1),
                                    512 * cc:512 * (cc + 1)],
                            in_=stgc[:])
                    return go
                for sq in sqs:
                    for cc in range(2):
                        fillers.append(mk(sq, cc))

            sp_t = {}
            pt_t = {}
            st_o = {}
            st_z = {}

            def emit_S(g, z):
                u, s = divmod(g, 16)
                a, hf = UNITS[u]
                zs = slice(64 * z, 64 * z + 64)
                sp = psS.tile([128, 1024], F32, tag=f"s{z}",
                              name=f"ps{g}_{z}")
                sp_t[(g, z)] = sp
                for qc in range(2):
                    nc.tensor.matmul(
                        sp[:, 512 * qc:512 * (qc + 1)],
                        AK[zs, a, 128 * s:128 * s + 128],
                        AQ[zs, a, 1024 * hf + 512 * qc:
                           1024 * hf + 512 * qc + 512],
                        start=True, stop=True)

            def emit_exp(g, z):
                pt = pt_pool.tile([128, 1024], BF16, tag="pt",
                                  name=f"pt{g}_{z}")
                pt_t[(g, z)] = pt
                nc.scalar.activation(pt[:], sp_t.pop((g, z))[:], EXPF,
                                     scale=0.125)

            def emit_PV(g):
                u, s = divmod(g, 16)
                a, hf = UNITS[u]
                if s == 0:
                    st_o[u] = psO.tile([128, 1024], F32, tag="o",
                                       name=f"po{u}")
                    st_z[u] = psZ.tile([128, 512], F32, tag="z",
                                       name=f"pz{u}")
                op, zp = st_o[u], st_z[u]
                pts = [pt_t.pop((g, 0)), pt_t.pop((g, 1))]
                for qc in range(2):
                    for z in range(2):
                        l = 2 * a + z
                        for c in range(2):
                            nc.tensor.matmul(
                                op[64 * z + 32 * c:64 * z + 32 * c + 32,
                                   512 * qc:512 * (qc + 1)],
                                VN[l][:, 64 * s + 32 * c:64 * s + 32 * c + 32],
                                pts[z][:, 512 * qc:512 * (qc + 1)],
                                start=(s == 0), stop=(s == 15),
                                tile_position=(0, 64 * z + 32 * c))
                for z in range(2):
                    for qc in range(2):
                        r = 64 * z + 32 * qc
                        nc.tensor.matmul(
                            zp[r:r + 1, :], onescol[:, 0:1],
                            pts[z][:, 512 * qc:512 * (qc + 1)],
                            start=(s == 0), stop=(s == 15),
                            tile_position=(0, r))

            def emit_norm(u):
                a, hf = UNITS[u]
                op, zp = st_o.pop(u), st_z.pop(u)
                with nc.allow_low_precision(
                        reason="softmax denom reciprocal in bf16: ~0.4% "
                               "rel, inside the 2e-2 gate"):
                    nc.vector.reciprocal(recZ[0:97, :], zp[0:97, :])
                for qc in range(2):
                    bc = psC.tile([128, 512], F32, tag="c", name=f"bc{u}{qc}")
                    for z in range(2):
                        r = 64 * z + 32 * qc
                        nc.tensor.matmul(
                            bc[64 * z:64 * z + 64, :],
                            onesb[r:r + 1, :], recZ[r:r + 1, :],
                            start=True, stop=True,
                            tile_position=(r, 64 * z))
                    bcs = stg_pool.tile([128, 512], BF16, tag="bcs",
                                        name=f"bcs{u}{qc}")
                    nc.vector.tensor_copy(bcs[:], bc[:])
                    nc.vector.tensor_mul(
                        ONP[a][:, 1024 * hf + 512 * qc:
                               1024 * hf + 512 * qc + 512],
                        op[:, 512 * qc:512 * (qc + 1)], bcs[:])

            # filler queue in deadline order
            make_projv_fillers(0, 1)
            make_projv_fillers(1, 1)
            for m2 in range(PRO_K, 8):
                make_projqk_fillers("k", m2, bk_sb, KK5)
            make_projv_fillers(2, 0)
            make_projv_fillers(3, 0)
            make_projv_fillers(2, 1)
            make_projv_fillers(3, 1)
            for m2 in range(PRO_Q, 8):
                make_projqk_fillers("q", m2, bq_sb, QQ5)

            def pop_fillers(n):
                for _ in range(n):
                    if fillers:
                        fillers.pop(0)()

            emit_S(0, 0)
            emit_S(0, 1)
            for g in range(NG):
                u, s = divmod(g, 16)
                if g == 36:
                    make_outproj_fillers(range(8))
                if g > 0:
                    emit_PV(g - 1)
                    if s == 0:
                        emit_norm(u - 1)
                emit_exp(g, 0)
                pop_fillers(2 if g < 24 else 1)
                if g + 1 < NG:
                    emit_S(g + 1, 0)
                pop_fillers(2 if s not in (15, 0) else 0)
                emit_exp(g, 1)
                if g + 1 < NG:
                    emit_S(g + 1, 1)
            emit_PV(NG - 1)
            emit_norm(3)
            make_outproj_fillers(range(8, 16), use_act=True)
            while fillers:
                fillers.pop(0)()


_NC_CACHE = None


def _get_program():
    global _NC_CACHE
    if _NC_CACHE is None:
        _NC_CACHE = build_program()
    return _NC_CACHE


def _prep_host(x, wq, bq, wk, bk, wv, bv, wo, bo, cos, sin):
    f32 = np.float32
    bf = ml_dtypes.bfloat16
    x = np.asarray(x, f32)
    wq, wk, wv, wo = (np.asarray(a, f32) for a in (wq, wk, wv, wo))
    bq, bk, bv, bo = (np.asarray(a, f32) for a in (bq, bk, bv, bo))
    cos, sin = np.asarray(cos, f32), np.asarray(sin, f32)

    # RoPE at fixed position T (reference bug, replicated): fold into weights.
    c_row = cos[T]
    s_row = sin[T]
    Cv = np.tile(c_row, H)
    Sv = np.tile(s_row, H)
    sgn = np.where(np.arange(DM) % 2 == 0, -1.0, 1.0).astype(f32)
    Ss = (sgn * Sv).astype(f32)
    swap = np.arange(DM) ^ 1

    wq_rot = Cv[:, None] * wq + Ss[:, None] * wq[swap, :]
    wk_rot = Cv[:, None] * wk + Ss[:, None] * wk[swap, :]
    bq_rot = Cv * bq + Ss * bq[swap]
    bk_rot = Cv * bk + Ss * bk[swap]

    wqTc = np.ascontiguousarray(wq_rot.T).reshape(8, 128, DM).astype(bf)
    wkTc = np.ascontiguousarray(wk_rot.T).reshape(8, 128, DM).astype(bf)
    wvTc = np.ascontiguousarray(wv.T).reshape(8, 128, DM).astype(bf)
    bqp = np.ascontiguousarray(bq_rot.reshape(8, 128).T).astype(f32)
    bkp = np.ascontiguousarray(bk_rot.reshape(8, 128).T).astype(f32)

    in_maps = []
    for i in range(N_CORES):
        b, j = i // 4, i % 4
        xT = x[b, RB * j:RB * (j + 1), :].T  # [1024, 512]
        xTp = np.ascontiguousarray(
            xT.reshape(8, 128, RB).transpose(1, 0, 2).reshape(128, 8 * RB)
        ).astype(bf)
        wopc = np.stack([
            np.ascontiguousarray(
                wo[:, 256 * j + 128 * a:256 * j + 128 * (a + 1)].T)
            for a in range(2)
        ]).astype(bf)
        in_maps.append({
            "xTp": xTp, "wqT": wqTc, "wkT": wkTc, "wvT": wvTc, "wop": wopc,
            "bqp": bqp, "bkp": bkp, "bvr": bv.reshape(1, DM),
            "ones1": np.ones((1, 128), f32),
        })
    return in_maps, bo


def kernel(x, wq, bq, wk, bk, wv, bv, wo, bo, cos, sin,
           _trace=False, _trace_kwargs=None):
    nc = _get_program()
    in_maps, bo_np = _prep_host(x, wq, bq, wk, bk, wv, bv, wo, bo, cos, sin)
    kw = {}
    if _trace:
        kw["trace"] = True
        if _trace_kwargs:
            kw.update(_trace_kwargs)
    res = run_bass_kernel_spmd(nc, in_maps, core_ids=list(range(N_CORES)), **kw)
    outf = np.zeros((B, T, DM), np.float32)
    for i in range(N_CORES):
        part = res.results[i]["out"].astype(np.float32)
        # rows arrive as pi = mp*256 + w*128 + tt; t = tt*16 + 2*mp + w
        part = part.reshape(8, 2, 128, DM).transpose(2, 0, 1, 3).reshape(T, DM)
        outf[i // 4] += part
    outf += bo_np[None, None, :]
    kernel.last_results = res
    return outf


# revision 18
# speedup vs baseline: 1.0086x; 1.0086x over previous
"""Trainium2 Bass kernel for nn_MultiHeadSelfAttention_65025804862080 (v4).

Full inputs in, full output out. Core i handles batch b=i//4 and heads
{4j..4j+3} (j=i%4). The reference's no-transpose head split means head h's
Q/K/V derive from x tokens [128h, 128h+128) only, so QKV shards across
cores with zero duplication. RoPE is position-independent (reference
indexes cos/sin at the single position t=T) and folds into wq/wk on host.

v4: pair-cycle schedule with PE array packing (see v3 notes) plus:
  - prologue runs p-outer so projection matmuls stream behind the weight
    DMAs; S(0) is emitted before the late prologue chunks so the exp
    stream starts as soon as K m2 0-3 / Q m2 0-3 drain.
  - exp table set preloaded via a dummy activation at t=0.
  - PV loads each V-chunk stationary once (qc innermost); Z ones-matmuls
    ride afterwards in the same 128x32 mode.
  - unit normalization split: reciprocal at s==0, broadcast+multiply at
    s==1, so the PE FIFO never waits on the DVE.
  - output projection: sq 0-7 fully in-loop; sq 8-15 pair-0 partials
    in-loop into SBUF, tail only runs the pair-1 matmul + DVE add + DMA
    through a dedicated 3-bank psum ring.
Token permutation: pi = mp*256 + w*128 + tt (m = 2mp+w = x column chunk
of 64, tt = token row in the head's 128-row x block); host unpermutes.
"""

import sys

if "/opt/trn_rl_repo" not in sys.path:
    sys.path.insert(0, "/opt/trn_rl_repo")

from contextlib import ExitStack

import ml_dtypes
import numpy as np

import concourse.tile as tile
from concourse import bacc, mybir
from concourse.bass_utils import run_bass_kernel_spmd

B, T, DM, H, DH = 2, 2048, 1024, 16, 64
N_CORES = 8
HPC = 4
RB = 512
F32 = mybir.dt.float32
F32R = mybir.dt.float32r
BF16 = mybir.dt.bfloat16
EXPF = mybir.ActivationFunctionType.Exp
ADD = mybir.AluOpType.add

NG = 64
UNITS = [(0, 0), (1, 0), (0, 1), (1, 1)]  # (pair a, q-half hf)


def build_program():
    nc = bacc.Bacc(
        "TRN2", target_bir_lowering=False, debug=False, num_devices=N_CORES
    )

    xTp = nc.dram_tensor("xTp", [128, 8 * RB], BF16, kind="ExternalInput").ap()
    wqT = nc.dram_tensor("wqT", [8, 128, DM], BF16, kind="ExternalInput").ap()
    wkT = nc.dram_tensor("wkT", [8, 128, DM], BF16, kind="ExternalInput").ap()
    wvT = nc.dram_tensor("wvT", [8, 128, DM], BF16, kind="ExternalInput").ap()
    wop = nc.dram_tensor("wop", [2, 128, DM], BF16, kind="ExternalInput").ap()
    bqp = nc.dram_tensor("bqp", [128, 8], F32, kind="ExternalInput").ap()
    bkp = nc.dram_tensor("bkp", [128, 8], F32, kind="ExternalInput").ap()
    bvr = nc.dram_tensor("bvr", [1, DM], F32R, kind="ExternalInput").ap()
    ones1 = nc.dram_tensor("ones1", [1, 128], F32R, kind="ExternalInput").ap()
    out = nc.dram_tensor("out", [T, DM], BF16, kind="ExternalOutput").ap()

    with tile.TileContext(nc) as tc:
        _emit(nc, tc, xTp, wqT, wkT, wvT, wop, bqp, bkp, bvr, ones1, out)

    nc.compile()
    return nc


def _emit(nc, tc, xTp, wqT, wkT, wvT, wop, bqp, bkp, bvr, ones1, out):
    ctx = ExitStack()
    with ctx:
        singles = ctx.enter_context(tc.tile_pool(name="singles", bufs=1))
        big = ctx.enter_context(tc.tile_pool(name="big", bufs=1))
        w_pool = ctx.enter_context(tc.tile_pool(name="wts", bufs=24))
        pt_pool = ctx.enter_context(tc.tile_pool(name="pt", bufs=6))
        stg_pool = ctx.enter_context(tc.tile_pool(name="stg", bufs=4))
        stgc_pool = ctx.enter_context(tc.tile_pool(name="stgc", bufs=4))

        XT = big.tile([128, 8 * RB], BF16, tag="xt", name="XT")
        QQ = big.tile([128, 2 * T], BF16, tag="qq", name="QQ")
        KK = big.tile([128, 2 * T], BF16, tag="kk", name="KK")
        VN = [big.tile([128, 16 * 64], BF16, tag=f"vn{l}", name=f"vn{l}")
              for l in range(HPC)]
        ONP = [big.tile([128, T], BF16, tag=f"onp{a}", name=f"onp{a}")
               for a in range(2)]
        WOP = [big.tile([128, DM], BF16, tag=f"wop{a}", name=f"wopp{a}")
               for a in range(2)]
        recZ = big.tile([128, 512], BF16, tag="rcz", name="recZ")
        OP0 = big.tile([128, 16 * 512], BF16, tag="op0", name="OP0")

        bq_sb = singles.tile([128, 8], F32, tag="bq", name="bq_sb")
        bk_sb = singles.tile([128, 8], F32, tag="bk", name="bk_sb")
        bv_sb = singles.tile([1, DM], F32R, tag="bv", name="bv_sb")
        ones_r = singles.tile([1, 128], F32R, tag="ones", name="ones_r")
        onescol = singles.tile([128, 1], BF16, tag="onec", name="onescol")
        onesb = singles.tile([128, 64], BF16, tag="onesb", name="onesb")
        dmy = singles.tile([1, 4], BF16, tag="dmy", name="dmy")
        nc.vector.memset(onescol, 1.0)
        nc.vector.memset(onesb, 1.0)
        nc.vector.memset(dmy, 1.0)
        # preload the exp table set while DMAs stream
        nc.scalar.activation(dmy[:], dmy[:], EXPF)

        # ---- initial DMAs: x on sync; weights p-interleaved on scalar ----
        x3o = XT.rearrange("p (k c) -> p k c", c=RB)
        x3i = xTp.rearrange("p (k c) -> p k c", c=RB)
        W = {}
        for nm in ("q", "k", "v"):
            for p in range(8):
                W[(nm, p)] = w_pool.tile([128, DM], BF16, tag="w",
                                         name=f"w{nm}{p}")
        for p in range(8):
            nsplit = 2 if p < 2 else 1
            for h in range(nsplit):
                w = 128 // nsplit
                nc.sync.dma_start(out=x3o[w * h:w * (h + 1), p, :],
                                  in_=x3i[w * h:w * (h + 1), p, :])
        for p in range(8):
            nk = 2 if p < 2 else 1
            for h in range(nk):
                w = 128 // nk
                nc.scalar.dma_start(out=W[("k", p)][w * h:w * (h + 1), :],
                                    in_=wkT[p, w * h:w * (h + 1), :])
            nq = 2 if p < 2 else 1
            for h in range(nq):
                w = 128 // nq
                nc.scalar.dma_start(out=W[("q", p)][w * h:w * (h + 1), :],
                                    in_=wqT[p, w * h:w * (h + 1), :])
            if p == 1:
                nc.scalar.dma_start(out=bq_sb, in_=bqp)
                nc.scalar.dma_start(out=bk_sb, in_=bkp)
                nc.scalar.dma_start(out=bv_sb, in_=bvr)
                nc.scalar.dma_start(out=ones_r, in_=ones1)
            nc.scalar.dma_start(out=W[("v", p)], in_=wvT[p])

        QQ5 = QQ.rearrange("p (a mp w tt) -> p a mp w tt", a=2, mp=8, w=2)
        KK5 = KK.rearrange("p (a mp w tt) -> p a mp w tt", a=2, mp=8, w=2)

        def qk_drain(ps, m2, b_sb, d5):
            srcs = [ps[0:64, :].rearrange("p (a zz t) -> p zz a t", a=2, zz=2),
                    ps[64:128, :].rearrange("p (a zz t) -> p zz a t",
                                            a=2, zz=2)]
            for w in range(2):
                nc.vector.tensor_scalar(
                    out=d5[64 * w:64 * w + 64, :, m2, w, :],
                    in0=srcs[w][:, w, :, :],
                    scalar1=b_sb[64 * w:64 * w + 64, m2:m2 + 1],
                    scalar2=None, op0=ADD)
            stg = stg_pool.tile([128, 256], BF16, tag="stg", name=f"sg{m2}")
            st3 = stg.rearrange("p (a t) -> p a t", a=2)
            nc.vector.tensor_scalar(
                out=st3[0:64], in0=srcs[0][:, 1, :, :],
                scalar1=b_sb[0:64, m2:m2 + 1], scalar2=None, op0=ADD)
            nc.vector.tensor_scalar(
                out=st3[64:128], in0=srcs[1][:, 0, :, :],
                scalar1=b_sb[64:128, m2:m2 + 1], scalar2=None, op0=ADD)
            nc.sync.dma_start(out=d5[64:128, :, m2, 0, :], in_=st3[0:64])
            nc.sync.dma_start(out=d5[0:64, :, m2, 1, :], in_=st3[64:128])

        # ---------- prologue: p-outer K0-3 / Q0-3 ----------
        with tc.tile_pool(name="psA", bufs=1, space="PSUM") as psA:
            psq = [psA.tile([128, RB], F32, tag=f"A{i}", name=f"psq{i}")
                   for i in range(8)]
            for p in range(8):
                for m2 in range(4):
                    nc.tensor.matmul(
                        psq[m2][:], W[("k", p)][:, 128 * m2:128 * (m2 + 1)],
                        x3o[:, p, :], start=(p == 0), stop=(p == 7))
                for m2 in range(4):
                    nc.tensor.matmul(
                        psq[4 + m2][:],
                        W[("q", p)][:, 128 * m2:128 * (m2 + 1)],
                        x3o[:, p, :], start=(p == 0), stop=(p == 7))
            for m2 in range(4):
                qk_drain(psq[m2], m2, bk_sb, KK5)
                qk_drain(psq[4 + m2], m2, bq_sb, QQ5)

        nc.sync.dma_start(out=WOP[0], in_=wop[0])
        nc.scalar.dma_start(out=WOP[1], in_=wop[1])

        # ---------- main loop ----------
        fillers = []

        with (
            tc.tile_pool(name="psS", bufs=1, space="PSUM") as psS,
            tc.tile_pool(name="psO", bufs=1, space="PSUM") as psO,
            tc.tile_pool(name="psZ", bufs=1, space="PSUM") as psZ,
            tc.tile_pool(name="psC", bufs=1, space="PSUM") as psC,
        ):
            AQ = QQ.rearrange("p (a q) -> p a q", a=2)
            AK = KK.rearrange("p (a q) -> p a q", a=2)

            def emit_projqk(nm, m2, b_sb, d5):
                t = psC.tile([128, RB], F32, tag="c", name=f"f{nm}{m2}")
                for p in range(8):
                    nc.tensor.matmul(
                        t[:], W[(nm, p)][:, 128 * m2:128 * (m2 + 1)],
                        x3o[:, p, :], start=(p == 0), stop=(p == 7))
                qk_drain(t, m2, b_sb, d5)

            def emit_projv(l, cc):
                t = psC.tile([128, 512], F32, tag="c", name=f"fv{l}{cc}")
                for p in range(8):
                    nc.tensor.matmul(
                        t[:], x3o[:, p, 128 * l:128 * (l + 1)],
                        W[("v", p)][:, 512 * cc:512 * (cc + 1)],
                        start=(p == 0), stop=False)
                nc.tensor.matmul(
                    t[:], ones_r[0:1, 0:128],
                    bv_sb[0:1, 512 * cc:512 * (cc + 1)],
                    start=False, stop=True)
                nc.vector.tensor_copy(VN[l][:, 512 * cc:512 * (cc + 1)], t[:])

            # fillers: list of CHUNKS (list of closures). A multi-closure
            # chunk owns the psC bank from first to last closure, so it must
            # never straddle a point where norm_b/outproj also allocate psC
            # (PE-FIFO vs psum-WAR inversion = deadlock). pop_fillers only
            # starts a multi-closure chunk early enough in the unit.
            def make_projqk_fillers(nm, m2, b_sb, d5):
                st = {}
                chunk = []

                def mk(p):
                    def go():
                        if p == 0:
                            st["t"] = psC.tile([128, RB], F32, tag="c",
                                               name=f"f{nm}{m2}")
                        nc.tensor.matmul(
                            st["t"][:],
                            W[(nm, p)][:, 128 * m2:128 * (m2 + 1)],
                            x3o[:, p, :], start=(p == 0), stop=(p == 7))
                        if p == 7:
                            qk_drain(st["t"], m2, b_sb, d5)
                    return go
                for p in range(8):
                    chunk.append(mk(p))
                fillers.append(chunk)

            def make_projv_fillers(l, cc):
                st = {}
                chunk = []

                def mk(p):
                    def go():
                        if p == 0:
                            st["t"] = psC.tile([128, 512], F32, tag="c",
                                               name=f"fv{l}{cc}")
                        nc.tensor.matmul(
                            st["t"][:], x3o[:, p, 128 * l:128 * (l + 1)],
                            W[("v", p)][:, 512 * cc:512 * (cc + 1)],
                            start=(p == 0), stop=False)
                        if p == 7:
                            nc.tensor.matmul(
                                st["t"][:], ones_r[0:1, 0:128],
                                bv_sb[0:1, 512 * cc:512 * (cc + 1)],
                                start=False, stop=True)
                            nc.vector.tensor_copy(
                                VN[l][:, 512 * cc:512 * (cc + 1)], st["t"][:])
                    return go
                for p in range(8):
                    chunk.append(mk(p))
                fillers.append(chunk)

            def make_outproj_full(sqs):
                def mk(sq, cc):
                    def go():
                        oc = psC.tile([128, 512], F32, tag="c",
                                      name=f"oc{sq}{cc}")
                        for a in range(2):
                            nc.tensor.matmul(
                                oc[:], ONP[a][:, 128 * sq:128 * sq + 128],
                                WOP[a][:, 512 * cc:512 * (cc + 1)],
                                start=(a == 0), stop=(a == 1))
                        stgc = stgc_pool.tile([128, 512], BF16, tag="sc",
                                              name=f"sc{sq}{cc}")
                        nc.vector.tensor_copy(stgc[:], oc[:])
                        eng = nc.sync if (sq + cc) % 2 == 0 else nc.gpsimd
                        eng.dma_start(
                            out=out[128 * sq:128 * (sq + 1),
                                    512 * cc:512 * (cc + 1)],
                            in_=stgc[:])
                    return go
                for sq in sqs:
                    for cc in range(2):
                        fillers.append([mk(sq, cc)])

            def make_outproj_a0(sqs):
                def mk(sq, cc):
                    def go():
                        oc = psC.tile([128, 512], F32, tag="c",
                                      name=f"pa{sq}{cc}")
                        nc.tensor.matmul(
                            oc[:], ONP[0][:, 128 * sq:128 * sq + 128],
                            WOP[0][:, 512 * cc:512 * (cc + 1)],
                            start=True, stop=True)
                        nc.vector.tensor_copy(
                            OP0[:, 512 * (2 * (sq - 8) + cc):
                                512 * (2 * (sq - 8) + cc) + 512], oc[:])
                    return go
                for sq in sqs:
                    for cc in range(2):
                        fillers.append([mk(sq, cc)])

            sp_t = {}
            pt_t = {}
            st_o = {}
            st_z = {}

            def emit_S(g, z):
                u, s = divmod(g, 16)
                a, hf = UNITS[u]
                zs = slice(64 * z, 64 * z + 64)
                sp = psS.tile([128, 1024], F32, tag=f"s{z}",
                              name=f"ps{g}_{z}")
                sp_t[(g, z)] = sp
                for qc in range(2):
                    nc.tensor.matmul(
                        sp[:, 512 * qc:512 * (qc + 1)],
                        AK[zs, a, 128 * s:128 * s + 128],
                        AQ[zs, a, 1024 * hf + 512 * qc:
                           1024 * hf + 512 * qc + 512],
                        start=True, stop=True)

            def emit_exp(g, z):
                pt = pt_pool.tile([128, 1024], BF16, tag="pt",
                                  name=f"pt{g}_{z}")
                pt_t[(g, z)] = pt
                nc.scalar.activation(pt[:], sp_t.pop((g, z))[:], EXPF,
                                     scale=0.125)

            def emit_PV(g):
                u, s = divmod(g, 16)
                a, hf = UNITS[u]
                if s == 0:
                    st_o[u] = psO.tile([128, 1024], F32, tag="o",
                                       name=f"po{u}")
                    st_z[u] = psZ.tile([128, 512], F32, tag="z",
                                       name=f"pz{u}")
                op, zp = st_o[u], st_z[u]
                pts = [pt_t.pop((g, 0)), pt_t.pop((g, 1))]
                for qc in range(2):
                    for z in range(2):
                        l = 2 * a + z
                        for c in range(2):
                            nc.tensor.matmul(
                                op[64 * z + 32 * c:64 * z + 32 * c + 32,
                                   512 * qc:512 * (qc + 1)],
                                VN[l][:, 64 * s + 32 * c:64 * s + 32 * c + 32],
                                pts[z][:, 512 * qc:512 * (qc + 1)],
                                start=(s == 0), stop=(s == 15),
                                tile_position=(0, 64 * z + 32 * c))
                for z in range(2):
                    for qc in range(2):
                        r = 64 * z + 32 * qc
                        nc.tensor.matmul(
                            zp[r:r + 1, :], onescol[:, 0:1],
                            pts[z][:, 512 * qc:512 * (qc + 1)],
                            start=(s == 0), stop=(s == 15),
                            tile_position=(0, r))

            def emit_norm_a(u):
                zp = st_z.pop(u)
                with nc.allow_low_precision(
                        reason="softmax denom reciprocal in bf16: ~0.4% "
                               "rel, inside the 2e-2 gate"):
                    nc.vector.reciprocal(recZ[0:97, :], zp[0:97, :])

            def emit_norm_b(u):
                a, hf = UNITS[u]
                op = st_o.pop(u)
                for qc in range(2):
                    bc = psC.tile([128, 512], F32, tag="c", name=f"bc{u}{qc}")
                    for z in range(2):
                        r = 64 * z + 32 * qc
                        nc.tensor.matmul(
                            bc[64 * z:64 * z + 64, :],
                            onesb[r:r + 1, :], recZ[r:r + 1, :],
                            start=True, stop=True,
                            tile_position=(r, 64 * z))
                    bcs = stg_pool.tile([128, 512], BF16, tag="bcs",
                                        name=f"bcs{u}{qc}")
                    nc.vector.tensor_copy(bcs[:], bc[:])
                    nc.vector.tensor_mul(
                        ONP[a][:, 1024 * hf + 512 * qc:
                               1024 * hf + 512 * qc + 512],
                        op[:, 512 * qc:512 * (qc + 1)], bcs[:])

            # filler queue with EMISSION deadlines: a producer chunk must be
            # fully emitted before the cycle whose S/PV emission reads its
            # output, else the consumer silently reads garbage (deps only
            # point backwards in program order).
            make_projv_fillers(0, 1)
            make_projv_fillers(1, 1)
            for m2 in range(6, 8):
                make_projqk_fillers("k", m2, bk_sb, KK5)
            make_projv_fillers(2, 0)
            make_projv_fillers(3, 0)
            make_projv_fillers(2, 1)
            make_projv_fillers(3, 1)
            for m2 in range(4, 8):
                make_projqk_fillers("q", m2, bq_sb, QQ5)
            DUE = [8, 8, 2 * 6 - 2, 2 * 7 - 2, 16, 16, 24, 24,
                   30, 30, 30, 30]
            dues = {id(ch): d for ch, d in zip(fillers, DUE)}

            cur_chunk = []

            def force_due(g):
                while fillers and dues.get(id(fillers[0]), 9999) <= g:
                    while cur_chunk:
                        cur_chunk.pop(0)()
                    cur_chunk.extend(fillers.pop(0))
                    while cur_chunk:
                        cur_chunk.pop(0)()

            def pop_fillers(n, s):
                for _ in range(n):
                    if not cur_chunk:
                        if not fillers:
                            return
                        if len(fillers[0]) > 1 and (s >= 14 or s == 0):
                            return  # don't start a chunk near a boundary
                        cur_chunk.extend(fillers.pop(0))
                    cur_chunk.pop(0)()

            # S(0) first so exps start the moment K0/Q0-3 drain; the late
            # prologue chunks stream behind it in the PE FIFO.
            emit_S(0, 0)
            emit_S(0, 1)
            emit_projv(0, 0)
            emit_projv(1, 0)
            emit_projqk("k", 4, bk_sb, KK5)
            emit_projqk("k", 5, bk_sb, KK5)

            for g in range(NG):
                u, s = divmod(g, 16)
                force_due(g)
                if g == 35:
                    make_outproj_full(range(8))
                if g == 50:
                    make_outproj_a0(range(8, 16))
                if g > 0:
                    # norm_b reads the previous unit's psO tile; it must be
                    # emitted BEFORE emit_PV(g-1) reallocates that ring slot
                    # at s==1 (use-after-realloc inverts the psum WAR).
                    if s == 1 and u > 0:
                        emit_norm_b(u - 1)
                    emit_PV(g - 1)
                    if s == 0 and u > 0:
                        emit_norm_a(u - 1)
                emit_exp(g, 0)
                pop_fillers(2 if g < 31 else 1, s)
                if g + 1 < NG:
                    emit_S(g + 1, 0)
                pop_fillers(2 if 1 < s else 1, s)
                emit_exp(g, 1)
                if g + 1 < NG:
                    emit_S(g + 1, 1)
            emit_PV(NG - 1)
            emit_norm_a(3)
            emit_norm_b(3)
            while fillers or cur_chunk:
                pop_fillers(1, 5)

        # ---------- tail: sq 8-15 = pair-1 matmul + add of pair-0 partial
        with tc.tile_pool(name="psT", bufs=3, space="PSUM") as psT:
            for sq in range(8, 16):
                for cc in range(2):
                    oc = psT.tile([128, 512], F32, tag="t",
                                  name=f"tl{sq}{cc}")
                    nc.tensor.matmul(
                        oc[:], ONP[1][:, 128 * sq:128 * sq + 128],
                        WOP[1][:, 512 * cc:512 * (cc + 1)],
                        start=True, stop=True)
                    stgc = stgc_pool.tile([128, 512], BF16, tag="sc",
                                          name=f"tc{sq}{cc}")
                    k = 2 * (sq - 8) + cc
                    if (sq + cc) % 2 == 0:
                        nc.vector.tensor_add(
                            stgc[:], oc[:], OP0[:, 512 * k:512 * k + 512])
                    else:
                        nc.vector.tensor_add(
                            stgc[:], oc[:], OP0[:, 512 * k:512 * k + 512])
                    eng = (nc.sync, nc.gpsimd, nc.scalar)[(sq + cc) % 3]
                    eng.dma_start(
                        out=out[128 * sq:128 * (sq + 1),
                                512 * cc:512 * (cc + 1)],
                        in_=stgc[:])


_NC_CACHE = None


def _get_program():
    global _NC_CACHE
    if _NC_CACHE is None:
        _NC_CACHE = build_program()
    return _NC_CACHE


def _prep_host(x, wq, bq, wk, bk, wv, bv, wo, bo, cos, sin):
    f32 = np.float32
    bf = ml_dtypes.bfloat16
    x = np.asarray(x, f32)
    wq, wk, wv, wo = (np.asarray(a, f32) for a in (wq, wk, wv, wo))
    bq, bk, bv, bo = (np.asarray(a, f32) for a in (bq, bk, bv, bo))
    cos, sin = np.asarray(cos, f32), np.asarray(sin, f32)

    c_row = cos[T]
    s_row = sin[T]
    Cv = np.tile(c_row, H)
    Sv = np.tile(s_row, H)
    sgn = np.where(np.arange(DM) % 2 == 0, -1.0, 1.0).astype(f32)
    Ss = (sgn * Sv).astype(f32)
    swap = np.arange(DM) ^ 1

    wq_rot = Cv[:, None] * wq + Ss[:, None] * wq[swap, :]
    wk_rot = Cv[:, None] * wk + Ss[:, None] * wk[swap, :]
    bq_rot = Cv * bq + Ss * bq[swap]
    bk_rot = Cv * bk + Ss * bk[swap]

    wqTc = np.ascontiguousarray(wq_rot.T).reshape(8, 128, DM).astype(bf)
    wkTc = np.ascontiguousarray(wk_rot.T).reshape(8, 128, DM).astype(bf)
    wvTc = np.ascontiguousarray(wv.T).reshape(8, 128, DM).astype(bf)
    bqp = np.ascontiguousarray(bq_rot.reshape(8, 128).T).astype(f32)
    bkp = np.ascontiguousarray(bk_rot.reshape(8, 128).T).astype(f32)

    in_maps = []
    for i in range(N_CORES):
        b, j = i // 4, i % 4
        xT = x[b, RB * j:RB * (j + 1), :].T
        xTp = np.ascontiguousarray(
            xT.reshape(8, 128, RB).transpose(1, 0, 2).reshape(128, 8 * RB)
        ).astype(bf)
        wopc = np.stack([
            np.ascontiguousarray(
                wo[:, 256 * j + 128 * a:256 * j + 128 * (a + 1)].T)
            for a in range(2)
        ]).astype(bf)
        in_maps.append({
            "xTp": xTp, "wqT": wqTc, "wkT": wkTc, "wvT": wvTc, "wop": wopc,
            "bqp": bqp, "bkp": bkp, "bvr": bv.reshape(1, DM),
            "ones1": np.ones((1, 128), f32),
        })
    return in_maps, bo


def kernel(x, wq, bq, wk, bk, wv, bv, wo, bo, cos, sin,
           _trace=False, _trace_kwargs=None):
    nc = _get_program()
    in_maps, bo_np = _prep_host(x, wq, bq, wk, bk, wv, bv, wo, bo, cos, sin)
    kw = {}
    if _trace:
        kw["trace"] = True
        if _trace_kwargs:
            kw.update(_trace_kwargs)
    res = run_bass_kernel_spmd(nc, in_maps, core_ids=list(range(N_CORES)), **kw)
    outf = np.zeros((B, T, DM), np.float32)
    for i in range(N_CORES):
        part = res.results[i]["out"].astype(np.float32)
        # rows arrive as pi = mp*256 + w*128 + tt; t = tt*16 + 2*mp + w
        part = part.reshape(8, 2, 128, DM).transpose(2, 0, 1, 3).reshape(T, DM)
        outf[i // 4] += part
    outf += bo_np[None, None, :]
    kernel.last_results = res
    return outf


# revision 20
# speedup vs baseline: 1.0165x; 1.0078x over previous
"""Trainium2 Bass kernel for nn_MultiHeadSelfAttention_65025804862080 (v4).

Full inputs in, full output out. Core i handles batch b=i//4 and heads
{4j..4j+3} (j=i%4). The reference's no-transpose head split means head h's
Q/K/V derive from x tokens [128h, 128h+128) only, so QKV shards across
cores with zero duplication. RoPE is position-independent (reference
indexes cos/sin at the single position t=T) and folds into wq/wk on host.

v4: pair-cycle schedule with PE array packing (see v3 notes) plus:
  - prologue runs p-outer so projection matmuls stream behind the weight
    DMAs; S(0) is emitted before the late prologue chunks so the exp
    stream starts as soon as K m2 0-3 / Q m2 0-3 drain.
  - exp table set preloaded via a dummy activation at t=0.
  - PV loads each V-chunk stationary once (qc innermost); Z ones-matmuls
    ride afterwards in the same 128x32 mode.
  - unit normalization split: reciprocal at s==0, broadcast+multiply at
    s==1, so the PE FIFO never waits on the DVE.
  - output projection: sq 0-7 fully in-loop; sq 8-15 pair-0 partials
    in-loop into SBUF, tail only runs the pair-1 matmul + DVE add + DMA
    through a dedicated 3-bank psum ring.
Token permutation: pi = mp*256 + w*128 + tt (m = 2mp+w = x column chunk
of 64, tt = token row in the head's 128-row x block); host unpermutes.
"""

import sys

if "/opt/trn_rl_repo" not in sys.path:
    sys.path.insert(0, "/opt/trn_rl_repo")

from contextlib import ExitStack

import ml_dtypes
import numpy as np

import concourse.tile as tile
from concourse import bacc, mybir
from concourse.bass_utils import run_bass_kernel_spmd

B, T, DM, H, DH = 2, 2048, 1024, 16, 64
N_CORES = 8
HPC = 4
RB = 512
F32 = mybir.dt.float32
F32R = mybir.dt.float32r
BF16 = mybir.dt.bfloat16
EXPF = mybir.ActivationFunctionType.Exp
ADD = mybir.AluOpType.add

NG = 64
UNITS = [(0, 0), (1, 0), (0, 1), (1, 1)]  # (pair a, q-half hf)


def build_program():
    nc = bacc.Bacc(
        "TRN2", target_bir_lowering=False, debug=False, num_devices=N_CORES
    )

    xTp = nc.dram_tensor("xTp", [128, 8 * RB], BF16, kind="ExternalInput").ap()
    wqT = nc.dram_tensor("wqT", [8, 128, DM], BF16, kind="ExternalInput").ap()
    wkT = nc.dram_tensor("wkT", [8, 128, DM], BF16, kind="ExternalInput").ap()
    wvT = nc.dram_tensor("wvT", [8, 128, DM], BF16, kind="ExternalInput").ap()
    wop = nc.dram_tensor("wop", [2, 128, DM], BF16, kind="ExternalInput").ap()
    bqp = nc.dram_tensor("bqp", [128, 8], F32, kind="ExternalInput").ap()
    bkp = nc.dram_tensor("bkp", [128, 8], F32, kind="ExternalInput").ap()
    bvr = nc.dram_tensor("bvr", [1, DM], F32R, kind="ExternalInput").ap()
    ones1 = nc.dram_tensor("ones1", [1, 128], F32R, kind="ExternalInput").ap()
    out = nc.dram_tensor("out", [T, DM], BF16, kind="ExternalOutput").ap()

    with tile.TileContext(nc) as tc:
        _emit(nc, tc, xTp, wqT, wkT, wvT, wop, bqp, bkp, bvr, ones1, out)

    nc.compile()
    return nc


def _emit(nc, tc, xTp, wqT, wkT, wvT, wop, bqp, bkp, bvr, ones1, out):
    ctx = ExitStack()
    with ctx:
        singles = ctx.enter_context(tc.tile_pool(name="singles", bufs=1))
        big = ctx.enter_context(tc.tile_pool(name="big", bufs=1))
        w_pool = ctx.enter_context(tc.tile_pool(name="wts", bufs=24))
        pt_pool = ctx.enter_context(tc.tile_pool(name="pt", bufs=6))
        stg_pool = ctx.enter_context(tc.tile_pool(name="stg", bufs=4))
        stgc_pool = ctx.enter_context(tc.tile_pool(name="stgc", bufs=4))

        XT = big.tile([128, 8 * RB], BF16, tag="xt", name="XT")
        QQ = big.tile([128, 2 * T], BF16, tag="qq", name="QQ")
        KK = big.tile([128, 2 * T], BF16, tag="kk", name="KK")
        VN = [big.tile([128, 16 * 64], BF16, tag=f"vn{l}", name=f"vn{l}")
              for l in range(HPC)]
        ONP = [big.tile([128, T], BF16, tag=f"onp{a}", name=f"onp{a}")
               for a in range(2)]
        WOP = [big.tile([128, DM], BF16, tag=f"wop{a}", name=f"wopp{a}")
               for a in range(2)]
        recZ = big.tile([128, 512], BF16, tag="rcz", name="recZ")
        OP0 = big.tile([128, 16 * 512], BF16, tag="op0", name="OP0")

        bq_sb = singles.tile([128, 8], F32, tag="bq", name="bq_sb")
        bk_sb = singles.tile([128, 8], F32, tag="bk", name="bk_sb")
        bv_sb = singles.tile([1, DM], F32R, tag="bv", name="bv_sb")
        ones_r = singles.tile([1, 128], F32R, tag="ones", name="ones_r")
        onescol = singles.tile([128, 1], BF16, tag="onec", name="onescol")
        onesb = singles.tile([128, 64], BF16, tag="onesb", name="onesb")
        dmy = singles.tile([1, 4], BF16, tag="dmy", name="dmy")
        nc.vector.memset(onescol, 1.0)
        nc.vector.memset(onesb, 1.0)
        nc.vector.memset(dmy, 1.0)
        # preload the exp table set while DMAs stream
        nc.scalar.activation(dmy[:], dmy[:], EXPF)

        # ---- initial DMAs: x on sync; weights p-interleaved on scalar ----
        x3o = XT.rearrange("p (k c) -> p k c", c=RB)
        x3i = xTp.rearrange("p (k c) -> p k c", c=RB)
        W = {}
        for nm in ("q", "k", "v"):
            for p in range(8):
                W[(nm, p)] = w_pool.tile([128, DM], BF16, tag="w",
                                         name=f"w{nm}{p}")
        # ring budget: scalar-ring triggers delay the first exp (the ACT
        # sequencer drains them first), so keep scalar to wq+biases only.
        # x + wk ride sync; wv + wop ride the gpsimd SWDGE queue.
        for p in range(8):
            nsplit = 2 if p < 2 else 1
            for h in range(nsplit):
                w = 128 // nsplit
                nc.sync.dma_start(out=x3o[w * h:w * (h + 1), p, :],
                                  in_=x3i[w * h:w * (h + 1), p, :])
        for p in range(8):
            nk = 2 if p < 2 else 1
            for h in range(nk):
                w = 128 // nk
                nc.sync.dma_start(out=W[("k", p)][w * h:w * (h + 1), :],
                                  in_=wkT[p, w * h:w * (h + 1), :])
            nq = 2 if p < 2 else 1
            for h in range(nq):
                w = 128 // nq
                nc.scalar.dma_start(out=W[("q", p)][w * h:w * (h + 1), :],
                                    in_=wqT[p, w * h:w * (h + 1), :])
            if p == 1:
                nc.scalar.dma_start(out=bq_sb, in_=bqp)
                nc.scalar.dma_start(out=bk_sb, in_=bkp)
                nc.scalar.dma_start(out=bv_sb, in_=bvr)
                nc.scalar.dma_start(out=ones_r, in_=ones1)
            nc.gpsimd.dma_start(out=W[("v", p)], in_=wvT[p])

        QQ5 = QQ.rearrange("p (a mp w tt) -> p a mp w tt", a=2, mp=8, w=2)
        KK5 = KK.rearrange("p (a mp w tt) -> p a mp w tt", a=2, mp=8, w=2)

        def qk_drain(ps, m2, b_sb, d5):
            srcs = [ps[0:64, :].rearrange("p (a zz t) -> p zz a t", a=2, zz=2),
                    ps[64:128, :].rearrange("p (a zz t) -> p zz a t",
                                            a=2, zz=2)]
            for w in range(2):
                nc.vector.tensor_scalar(
                    out=d5[64 * w:64 * w + 64, :, m2, w, :],
                    in0=srcs[w][:, w, :, :],
                    scalar1=b_sb[64 * w:64 * w + 64, m2:m2 + 1],
                    scalar2=None, op0=ADD)
            stg = stg_pool.tile([128, 256], BF16, tag="stg", name=f"sg{m2}")
            st3 = stg.rearrange("p (a t) -> p a t", a=2)
            nc.vector.tensor_scalar(
                out=st3[0:64], in0=srcs[0][:, 1, :, :],
                scalar1=b_sb[0:64, m2:m2 + 1], scalar2=None, op0=ADD)
            nc.vector.tensor_scalar(
                out=st3[64:128], in0=srcs[1][:, 0, :, :],
                scalar1=b_sb[64:128, m2:m2 + 1], scalar2=None, op0=ADD)
            nc.sync.dma_start(out=d5[64:128, :, m2, 0, :], in_=st3[0:64])
            nc.sync.dma_start(out=d5[0:64, :, m2, 1, :], in_=st3[64:128])

        # ---------- prologue: p-outer K0-3 / Q0-3 ----------
        with tc.tile_pool(name="psA", bufs=1, space="PSUM") as psA:
            psq = [psA.tile([128, RB], F32, tag=f"A{i}", name=f"psq{i}")
                   for i in range(8)]
            for p in range(8):
                for m2 in range(4):
                    nc.tensor.matmul(
                        psq[m2][:], W[("k", p)][:, 128 * m2:128 * (m2 + 1)],
                        x3o[:, p, :], start=(p == 0), stop=(p == 7))
                for m2 in range(4):
                    nc.tensor.matmul(
                        psq[4 + m2][:],
                        W[("q", p)][:, 128 * m2:128 * (m2 + 1)],
                        x3o[:, p, :], start=(p == 0), stop=(p == 7))
            for m2 in range(4):
                qk_drain(psq[m2], m2, bk_sb, KK5)
                qk_drain(psq[4 + m2], m2, bq_sb, QQ5)

        nc.sync.dma_start(out=WOP[0], in_=wop[0])
        nc.gpsimd.dma_start(out=WOP[1], in_=wop[1])

        # ---------- main loop ----------
        fillers = []

        with (
            tc.tile_pool(name="psS", bufs=1, space="PSUM") as psS,
            tc.tile_pool(name="psO", bufs=1, space="PSUM") as psO,
            tc.tile_pool(name="psZ", bufs=1, space="PSUM") as psZ,
            tc.tile_pool(name="psC", bufs=1, space="PSUM") as psC,
        ):
            AQ = QQ.rearrange("p (a q) -> p a q", a=2)
            AK = KK.rearrange("p (a q) -> p a q", a=2)

            def emit_projqk(nm, m2, b_sb, d5):
                t = psC.tile([128, RB], F32, tag="c", name=f"f{nm}{m2}")
                for p in range(8):
                    nc.tensor.matmul(
                        t[:], W[(nm, p)][:, 128 * m2:128 * (m2 + 1)],
                        x3o[:, p, :], start=(p == 0), stop=(p == 7))
                qk_drain(t, m2, b_sb, d5)

            def emit_projv(l, cc):
                t = psC.tile([128, 512], F32, tag="c", name=f"fv{l}{cc}")
                for p in range(8):
                    nc.tensor.matmul(
                        t[:], x3o[:, p, 128 * l:128 * (l + 1)],
                        W[("v", p)][:, 512 * cc:512 * (cc + 1)],
                        start=(p == 0), stop=False)
                nc.tensor.matmul(
                    t[:], ones_r[0:1, 0:128],
                    bv_sb[0:1, 512 * cc:512 * (cc + 1)],
                    start=False, stop=True)
                nc.vector.tensor_copy(VN[l][:, 512 * cc:512 * (cc + 1)], t[:])

            # fillers: list of CHUNKS (list of closures). A multi-closure
            # chunk owns the psC bank from first to last closure, so it must
            # never straddle a point where norm_b/outproj also allocate psC
            # (PE-FIFO vs psum-WAR inversion = deadlock). pop_fillers only
            # starts a multi-closure chunk early enough in the unit.
            def make_projqk_fillers(nm, m2, b_sb, d5):
                st = {}
                chunk = []

                def mk(p):
                    def go():
                        if p == 0:
                            st["t"] = psC.tile([128, RB], F32, tag="c",
                                               name=f"f{nm}{m2}")
                        nc.tensor.matmul(
                            st["t"][:],
                            W[(nm, p)][:, 128 * m2:128 * (m2 + 1)],
                            x3o[:, p, :], start=(p == 0), stop=(p == 7))
                        if p == 7:
                            qk_drain(st["t"], m2, b_sb, d5)
                    return go
                for p in range(8):
                    chunk.append(mk(p))
                fillers.append(chunk)

            def make_projv_fillers(l, cc):
                st = {}
                chunk = []

                def mk(p):
                    def go():
                        if p == 0:
                            st["t"] = psC.tile([128, 512], F32, tag="c",
                                               name=f"fv{l}{cc}")
                        nc.tensor.matmul(
                            st["t"][:], x3o[:, p, 128 * l:128 * (l + 1)],
                            W[("v", p)][:, 512 * cc:512 * (cc + 1)],
                            start=(p == 0), stop=False)
                        if p == 7:
                            nc.tensor.matmul(
                                st["t"][:], ones_r[0:1, 0:128],
                                bv_sb[0:1, 512 * cc:512 * (cc + 1)],
                                start=False, stop=True)
                            nc.vector.tensor_copy(
                                VN[l][:, 512 * cc:512 * (cc + 1)], st["t"][:])
                    return go
                for p in range(8):
                    chunk.append(mk(p))
                fillers.append(chunk)

            def make_outproj_full(sqs):
                def mk(sq, cc):
                    def go():
                        oc = psC.tile([128, 512], F32, tag="c",
                                      name=f"oc{sq}{cc}")
                        for a in range(2):
                            nc.tensor.matmul(
                                oc[:], ONP[a][:, 128 * sq:128 * sq + 128],
                                WOP[a][:, 512 * cc:512 * (cc + 1)],
                                start=(a == 0), stop=(a == 1))
                        stgc = stgc_pool.tile([128, 512], BF16, tag="sc",
                                              name=f"sc{sq}{cc}")
                        nc.vector.tensor_copy(stgc[:], oc[:])
                        eng = nc.sync if (sq + cc) % 2 == 0 else nc.gpsimd
                        eng.dma_start(
                            out=out[128 * sq:128 * (sq + 1),
                                    512 * cc:512 * (cc + 1)],
                            in_=stgc[:])
                    return go
                for sq in sqs:
                    for cc in range(2):
                        fillers.append([mk(sq, cc)])

            def make_outproj_a0(sqs):
                def mk(sq, cc):
                    def go():
                        oc = psC.tile([128, 512], F32, tag="c",
                                      name=f"pa{sq}{cc}")
                        nc.tensor.matmul(
                            oc[:], ONP[0][:, 128 * sq:128 * sq + 128],
                            WOP[0][:, 512 * cc:512 * (cc + 1)],
                            start=True, stop=True)
                        nc.vector.tensor_copy(
                            OP0[:, 512 * (2 * (sq - 8) + cc):
                                512 * (2 * (sq - 8) + cc) + 512], oc[:])
                    return go
                for sq in sqs:
                    for cc in range(2):
                        fillers.append([mk(sq, cc)])

            sp_t = {}
            pt_t = {}
            st_o = {}
            st_z = {}

            def emit_S(g, z):
                u, s = divmod(g, 16)
                a, hf = UNITS[u]
                zs = slice(64 * z, 64 * z + 64)
                sp = psS.tile([128, 1024], F32, tag=f"s{z}",
                              name=f"ps{g}_{z}")
                sp_t[(g, z)] = sp
                for qc in range(2):
                    nc.tensor.matmul(
                        sp[:, 512 * qc:512 * (qc + 1)],
                        AK[zs, a, 128 * s:128 * s + 128],
                        AQ[zs, a, 1024 * hf + 512 * qc:
                           1024 * hf + 512 * qc + 512],
                        start=True, stop=True)

            def emit_exp(g, z):
                pt = pt_pool.tile([128, 1024], BF16, tag="pt",
                                  name=f"pt{g}_{z}")
                pt_t[(g, z)] = pt
                nc.scalar.activation(pt[:], sp_t.pop((g, z))[:], EXPF,
                                     scale=0.125)

            def emit_PV(g):
                u, s = divmod(g, 16)
                a, hf = UNITS[u]
                if s == 0:
                    st_o[u] = psO.tile([128, 1024], F32, tag="o",
                                       name=f"po{u}")
                    st_z[u] = psZ.tile([128, 512], F32, tag="z",
                                       name=f"pz{u}")
                op, zp = st_o[u], st_z[u]
                pts = [pt_t.pop((g, 0)), pt_t.pop((g, 1))]
                for qc in range(2):
                    for z in range(2):
                        l = 2 * a + z
                        for c in range(2):
                            nc.tensor.matmul(
                                op[64 * z + 32 * c:64 * z + 32 * c + 32,
                                   512 * qc:512 * (qc + 1)],
                                VN[l][:, 64 * s + 32 * c:64 * s + 32 * c + 32],
                                pts[z][:, 512 * qc:512 * (qc + 1)],
                                start=(s == 0), stop=(s == 15),
                                tile_position=(0, 64 * z + 32 * c))
                for z in range(2):
                    for qc in range(2):
                        r = 64 * z + 32 * qc
                        nc.tensor.matmul(
                            zp[r:r + 1, :], onescol[:, 0:1],
                            pts[z][:, 512 * qc:512 * (qc + 1)],
                            start=(s == 0), stop=(s == 15),
                            tile_position=(0, r))

            def emit_norm_a(u):
                zp = st_z.pop(u)
                with nc.allow_low_precision(
                        reason="softmax denom reciprocal in bf16: ~0.4% "
                               "rel, inside the 2e-2 gate"):
                    nc.vector.reciprocal(recZ[0:97, :], zp[0:97, :])

            def emit_norm_b(u):
                a, hf = UNITS[u]
                op = st_o.pop(u)
                for qc in range(2):
                    bc = psC.tile([128, 512], F32, tag="c", name=f"bc{u}{qc}")
                    for z in range(2):
                        r = 64 * z + 32 * qc
                        nc.tensor.matmul(
                            bc[64 * z:64 * z + 64, :],
                            onesb[r:r + 1, :], recZ[r:r + 1, :],
                            start=True, stop=True,
                            tile_position=(r, 64 * z))
                    bcs = stg_pool.tile([128, 512], BF16, tag="bcs",
                                        name=f"bcs{u}{qc}")
                    nc.vector.tensor_copy(bcs[:], bc[:])
                    nc.vector.tensor_mul(
                        ONP[a][:, 1024 * hf + 512 * qc:
                               1024 * hf + 512 * qc + 512],
                        op[:, 512 * qc:512 * (qc + 1)], bcs[:])

            # filler queue with EMISSION deadlines: a producer chunk must be
            # fully emitted before the cycle whose S/PV emission reads its
            # output, else the consumer silently reads garbage (deps only
            # point backwards in program order).
            make_projv_fillers(0, 1)
            make_projv_fillers(1, 1)
            for m2 in range(6, 8):
                make_projqk_fillers("k", m2, bk_sb, KK5)
            make_projv_fillers(2, 0)
            make_projv_fillers(3, 0)
            make_projv_fillers(2, 1)
            make_projv_fillers(3, 1)
            for m2 in range(4, 8):
                make_projqk_fillers("q", m2, bq_sb, QQ5)
            DUE = [8, 8, 2 * 6 - 2, 2 * 7 - 2, 16, 16, 24, 24,
                   30, 30, 30, 30]
            dues = {id(ch): d for ch, d in zip(fillers, DUE)}

            cur_chunk = []

            def force_due(g):
                while fillers and dues.get(id(fillers[0]), 9999) <= g:
                    while cur_chunk:
                        cur_chunk.pop(0)()
                    cur_chunk.extend(fillers.pop(0))
                    while cur_chunk:
                        cur_chunk.pop(0)()

            def pop_fillers(n, s):
                for _ in range(n):
                    if not cur_chunk:
                        if not fillers:
                            return
                        if len(fillers[0]) > 1 and (s >= 14 or s == 0):
                            return  # don't start a chunk near a boundary
                        cur_chunk.extend(fillers.pop(0))
                    cur_chunk.pop(0)()

            # S(0) first so exps start the moment K0/Q0-3 drain; the late
            # prologue chunks stream behind it in the PE FIFO.
            emit_S(0, 0)
            emit_S(0, 1)
            emit_projv(0, 0)
            emit_projv(1, 0)
            emit_projqk("k", 4, bk_sb, KK5)
            emit_projqk("k", 5, bk_sb, KK5)

            for g in range(NG):
                u, s = divmod(g, 16)
                force_due(g)
                if g == 35:
                    make_outproj_full(range(8))
                if g == 50:
                    make_outproj_a0(range(8, 16))
                if g > 0:
                    # norm_b reads the previous unit's psO tile; it must be
                    # emitted BEFORE emit_PV(g-1) reallocates that ring slot
                    # at s==1 (use-after-realloc inverts the psum WAR).
                    if s == 1 and u > 0:
                        emit_norm_b(u - 1)
                    emit_PV(g - 1)
                    if s == 0 and u > 0:
                        emit_norm_a(u - 1)
                emit_exp(g, 0)
                pop_fillers(2 if g < 31 else 1, s)
                if g + 1 < NG:
                    emit_S(g + 1, 0)
                pop_fillers(2 if 1 < s else 1, s)
                emit_exp(g, 1)
                if g + 1 < NG:
                    emit_S(g + 1, 1)
            emit_PV(NG - 1)
            emit_norm_a(3)
            emit_norm_b(3)
            while fillers or cur_chunk:
                pop_fillers(1, 5)

        # ---------- tail: sq 8-15 = pair-1 matmul + add of pair-0 partial
        with tc.tile_pool(name="psT", bufs=3, space="PSUM") as psT:
            for sq in range(8, 16):
                for cc in range(2):
                    oc = psT.tile([128, 512], F32, tag="t",
                                  name=f"tl{sq}{cc}")
                    nc.tensor.matmul(
                        oc[:], ONP[1][:, 128 * sq:128 * sq + 128],
                        WOP[1][:, 512 * cc:512 * (cc + 1)],
                        start=True, stop=True)
                    stgc = stgc_pool.tile([128, 512], BF16, tag="sc",
                                          name=f"tc{sq}{cc}")
                    k = 2 * (sq - 8) + cc
                    if (sq + cc) % 2 == 0:
                        nc.vector.tensor_add(
                            stgc[:], oc[:], OP0[:, 512 * k:512 * k + 512])
                    else:
                        nc.vector.tensor_add(
                            stgc[:], oc[:], OP0[:, 512 * k:512 * k + 512])
                    eng = (nc.sync, nc.gpsimd, nc.scalar)[(sq + cc) % 3]
                    eng.dma_start(
                        out=out[128 * sq:128 * (sq + 1),
                                512 * cc:512 * (cc + 1)],
                        in_=stgc[:])


_NC_CACHE = None


def _get_program():
    global _NC_CACHE
    if _NC_CACHE is None:
        _NC_CACHE = build_program()
    return _NC_CACHE


def _prep_host(x, wq, bq, wk, bk, wv, bv, wo, bo, cos, sin):
    f32 = np.float32
    bf = ml_dtypes.bfloat16
    x = np.asarray(x, f32)
    wq, wk, wv, wo = (np.asarray(a, f32) for a in (wq, wk, wv, wo))
    bq, bk, bv, bo = (np.asarray(a, f32) for a in (bq, bk, bv, bo))
    cos, sin = np.asarray(cos, f32), np.asarray(sin, f32)

    c_row = cos[T]
    s_row = sin[T]
    Cv = np.tile(c_row, H)
    Sv = np.tile(s_row, H)
    sgn = np.where(np.arange(DM) % 2 == 0, -1.0, 1.0).astype(f32)
    Ss = (sgn * Sv).astype(f32)
    swap = np.arange(DM) ^ 1

    wq_rot = Cv[:, None] * wq + Ss[:, None] * wq[swap, :]
    wk_rot = Cv[:, None] * wk + Ss[:, None] * wk[swap, :]
    bq_rot = Cv * bq + Ss * bq[swap]
    bk_rot = Cv * bk + Ss * bk[swap]

    wqTc = np.ascontiguousarray(wq_rot.T).reshape(8, 128, DM).astype(bf)
    wkTc = np.ascontiguousarray(wk_rot.T).reshape(8, 128, DM).astype(bf)
    wvTc = np.ascontiguousarray(wv.T).reshape(8, 128, DM).astype(bf)
    bqp = np.ascontiguousarray(bq_rot.reshape(8, 128).T).astype(f32)
    bkp = np.ascontiguousarray(bk_rot.reshape(8, 128).T).astype(f32)

    in_maps = []
    for i in range(N_CORES):
        b, j = i // 4, i % 4
        xT = x[b, RB * j:RB * (j + 1), :].T
        xTp = np.ascontiguousarray(
            xT.reshape(8, 128, RB).transpose(1, 0, 2).reshape(128, 8 * RB)
        ).astype(bf)
        wopc = np.stack([
            np.ascontiguousarray(
                wo[:, 256 * j + 128 * a:256 * j + 128 * (a + 1)].T)
            for a in range(2)
        ]).astype(bf)
        in_maps.append({
            "xTp": xTp, "wqT": wqTc, "wkT": wkTc, "wvT": wvTc, "wop": wopc,
            "bqp": bqp, "bkp": bkp, "bvr": bv.reshape(1, DM),
            "ones1": np.ones((1, 128), f32),
        })
    return in_maps, bo


def kernel(x, wq, bq, wk, bk, wv, bv, wo, bo, cos, sin,
           _trace=False, _trace_kwargs=None):
    nc = _get_program()
    in_maps, bo_np = _prep_host(x, wq, bq, wk, bk, wv, bv, wo, bo, cos, sin)
    kw = {}
    if _trace:
        kw["trace"] = True
        if _trace_kwargs:
            kw.update(_trace_kwargs)
    res = run_bass_kernel_spmd(nc, in_maps, core_ids=list(range(N_CORES)), **kw)
    outf = np.zeros((B, T, DM), np.float32)
    for i in range(N_CORES):
        part = res.results[i]["out"].astype(np.float32)
        # rows arrive as pi = mp*256 + w*128 + tt; t = tt*16 + 2*mp + w
        part = part.reshape(8, 2, 128, DM).transpose(2, 0, 1, 3).reshape(T, DM)
        outf[i // 4] += part
    outf += bo_np[None, None, :]
    kernel.last_results = res
    return outf


# revision 25
# speedup vs baseline: 1.0283x; 1.0117x over previous
"""Trainium2 Bass kernel for nn_MultiHeadSelfAttention_65025804862080 (v4).

Full inputs in, full output out. Core i handles batch b=i//4 and heads
{4j..4j+3} (j=i%4). The reference's no-transpose head split means head h's
Q/K/V derive from x tokens [128h, 128h+128) only, so QKV shards across
cores with zero duplication. RoPE is position-independent (reference
indexes cos/sin at the single position t=T) and folds into wq/wk on host.

v4: pair-cycle schedule with PE array packing (see v3 notes) plus:
  - prologue runs p-outer so projection matmuls stream behind the weight
    DMAs; S(0) is emitted before the late prologue chunks so the exp
    stream starts as soon as K m2 0-3 / Q m2 0-3 drain.
  - exp table set preloaded via a dummy activation at t=0.
  - PV loads each V-chunk stationary once (qc innermost); Z ones-matmuls
    ride afterwards in the same 128x32 mode.
  - unit normalization split: reciprocal at s==0, broadcast+multiply at
    s==1, so the PE FIFO never waits on the DVE.
  - output projection: sq 0-7 fully in-loop; sq 8-15 pair-0 partials
    in-loop into SBUF, tail only runs the pair-1 matmul + DVE add + DMA
    through a dedicated 3-bank psum ring.
Token permutation: pi = mp*256 + w*128 + tt (m = 2mp+w = x column chunk
of 64, tt = token row in the head's 128-row x block); host unpermutes.
"""

import sys

if "/opt/trn_rl_repo" not in sys.path:
    sys.path.insert(0, "/opt/trn_rl_repo")

from contextlib import ExitStack

import ml_dtypes
import numpy as np

import concourse.tile as tile
from concourse import bacc, mybir
from concourse.bass_utils import run_bass_kernel_spmd

B, T, DM, H, DH = 2, 2048, 1024, 16, 64
N_CORES = 8
HPC = 4
RB = 512
F32 = mybir.dt.float32
F32R = mybir.dt.float32r
BF16 = mybir.dt.bfloat16
EXPF = mybir.ActivationFunctionType.Exp
ADD = mybir.AluOpType.add

NG = 64
UNITS = [(0, 0), (1, 0), (0, 1), (1, 1)]  # (pair a, q-half hf)


def build_program():
    nc = bacc.Bacc(
        "TRN2", target_bir_lowering=False, debug=False, num_devices=N_CORES
    )

    xTp = nc.dram_tensor("xTp", [128, 8 * RB], BF16, kind="ExternalInput").ap()
    wqT = nc.dram_tensor("wqT", [8, 128, DM], BF16, kind="ExternalInput").ap()
    wkT = nc.dram_tensor("wkT", [8, 128, DM], BF16, kind="ExternalInput").ap()
    wvT = nc.dram_tensor("wvT", [8, 128, DM], BF16, kind="ExternalInput").ap()
    wop = nc.dram_tensor("wop", [2, 128, DM], BF16, kind="ExternalInput").ap()
    bqp = nc.dram_tensor("bqp", [128, 8], F32, kind="ExternalInput").ap()
    bkp = nc.dram_tensor("bkp", [128, 8], F32, kind="ExternalInput").ap()
    bvr = nc.dram_tensor("bvr", [1, DM], F32R, kind="ExternalInput").ap()
    ones1 = nc.dram_tensor("ones1", [1, 128], F32R, kind="ExternalInput").ap()
    out = nc.dram_tensor("out", [T, DM], BF16, kind="ExternalOutput").ap()

    with tile.TileContext(nc) as tc:
        _emit(nc, tc, xTp, wqT, wkT, wvT, wop, bqp, bkp, bvr, ones1, out)

    nc.compile()
    return nc


def _emit(nc, tc, xTp, wqT, wkT, wvT, wop, bqp, bkp, bvr, ones1, out):
    ctx = ExitStack()
    with ctx:
        singles = ctx.enter_context(tc.tile_pool(name="singles", bufs=1))
        big = ctx.enter_context(tc.tile_pool(name="big", bufs=1))
        w_pool = ctx.enter_context(tc.tile_pool(name="wts", bufs=24))
        pt_pool = ctx.enter_context(tc.tile_pool(name="pt", bufs=6))
        stg_pool = ctx.enter_context(tc.tile_pool(name="stg", bufs=4))
        stgc_pool = ctx.enter_context(tc.tile_pool(name="stgc", bufs=4))

        XT = big.tile([128, 8 * RB], BF16, tag="xt", name="XT")
        QQ = big.tile([128, 2 * T], BF16, tag="qq", name="QQ")
        KK = big.tile([128, 2 * T], BF16, tag="kk", name="KK")
        VN = [big.tile([128, 16 * 64], BF16, tag=f"vn{l}", name=f"vn{l}")
              for l in range(HPC)]
        ONP = [big.tile([128, T], BF16, tag=f"onp{a}", name=f"onp{a}")
               for a in range(2)]
        WOP = [big.tile([128, DM], BF16, tag=f"wop{a}", name=f"wopp{a}")
               for a in range(2)]
        recZ = big.tile([128, 512], BF16, tag="rcz", name="recZ")
        OP0 = big.tile([128, 16 * 512], BF16, tag="op0", name="OP0")

        bq_sb = singles.tile([128, 8], F32, tag="bq", name="bq_sb")
        bk_sb = singles.tile([128, 8], F32, tag="bk", name="bk_sb")
        bv_sb = singles.tile([1, DM], F32R, tag="bv", name="bv_sb")
        ones_r = singles.tile([1, 128], F32R, tag="ones", name="ones_r")
        onescol = singles.tile([128, 1], BF16, tag="onec", name="onescol")
        onesb = singles.tile([128, 64], BF16, tag="onesb", name="onesb")
        dmy = singles.tile([1, 4], BF16, tag="dmy", name="dmy")
        nc.vector.memset(onescol, 1.0)
        nc.vector.memset(onesb, 1.0)
        nc.vector.memset(dmy, 1.0)
        # preload the exp table set while DMAs stream
        nc.scalar.activation(dmy[:], dmy[:], EXPF)

        # ---- initial DMAs: x on sync; weights p-interleaved on scalar ----
        x3o = XT.rearrange("p (k c) -> p k c", c=RB)
        x3i = xTp.rearrange("p (k c) -> p k c", c=RB)
        W = {}
        for nm in ("q", "k", "v"):
            for p in range(8):
                W[(nm, p)] = w_pool.tile([128, DM], BF16, tag="w",
                                         name=f"w{nm}{p}")
        # ring budget: each trigger is ~600ns of sequencer time, and the
        # prologue consumes chunks p-ordered, so interleave x/wk/wq
        # p-matched across the sync and vector rings (both otherwise idle
        # at start); wq+biases on scalar; wv+wop on the gpsimd SWDGE.
        for p in range(8):
            nc.sync.dma_start(out=x3o[:, p, :], in_=x3i[:, p, :])
            nc.sync.dma_start(out=W[("k", p)][0:64, :],
                              in_=wkT[p, 0:64, :])
            nc.sync.dma_start(out=W[("k", p)][64:128, :],
                              in_=wkT[p, 64:128, :])
            nq = 2 if p < 2 else 1
            for h in range(nq):
                w = 128 // nq
                nc.scalar.dma_start(out=W[("q", p)][w * h:w * (h + 1), :],
                                    in_=wqT[p, w * h:w * (h + 1), :])
            if p == 1:
                nc.scalar.dma_start(out=bq_sb, in_=bqp)
                nc.scalar.dma_start(out=bk_sb, in_=bkp)
                nc.scalar.dma_start(out=bv_sb, in_=bvr)
                nc.scalar.dma_start(out=ones_r, in_=ones1)
            nc.gpsimd.dma_start(out=W[("v", p)], in_=wvT[p])

        QQ5 = QQ.rearrange("p (a mp w tt) -> p a mp w tt", a=2, mp=8, w=2)
        KK5 = KK.rearrange("p (a mp w tt) -> p a mp w tt", a=2, mp=8, w=2)

        def qk_drain(ps, m2, b_sb, d5):
            srcs = [ps[0:64, :].rearrange("p (a zz t) -> p zz a t", a=2, zz=2),
                    ps[64:128, :].rearrange("p (a zz t) -> p zz a t",
                                            a=2, zz=2)]
            for w in range(2):
                nc.vector.tensor_scalar(
                    out=d5[64 * w:64 * w + 64, :, m2, w, :],
                    in0=srcs[w][:, w, :, :],
                    scalar1=b_sb[64 * w:64 * w + 64, m2:m2 + 1],
                    scalar2=None, op0=ADD)
            stg = stg_pool.tile([128, 256], BF16, tag="stg", name=f"sg{m2}")
            st3 = stg.rearrange("p (a t) -> p a t", a=2)
            nc.vector.tensor_scalar(
                out=st3[0:64], in0=srcs[0][:, 1, :, :],
                scalar1=b_sb[0:64, m2:m2 + 1], scalar2=None, op0=ADD)
            nc.vector.tensor_scalar(
                out=st3[64:128], in0=srcs[1][:, 0, :, :],
                scalar1=b_sb[64:128, m2:m2 + 1], scalar2=None, op0=ADD)
            nc.sync.dma_start(out=d5[64:128, :, m2, 0, :], in_=st3[0:64])
            nc.sync.dma_start(out=d5[0:64, :, m2, 1, :], in_=st3[64:128])

        # ---------- prologue: p-outer K0-3 / Q0-3 ----------
        with tc.tile_pool(name="psA", bufs=1, space="PSUM") as psA:
            psq = [psA.tile([128, RB], F32, tag=f"A{i}", name=f"psq{i}")
                   for i in range(8)]
            for p in range(8):
                for m2 in range(4):
                    nc.tensor.matmul(
                        psq[m2][:], W[("k", p)][:, 128 * m2:128 * (m2 + 1)],
                        x3o[:, p, :], start=(p == 0), stop=(p == 7))
                for m2 in range(4):
                    nc.tensor.matmul(
                        psq[4 + m2][:],
                        W[("q", p)][:, 128 * m2:128 * (m2 + 1)],
                        x3o[:, p, :], start=(p == 0), stop=(p == 7))
            for m2 in range(4):
                qk_drain(psq[m2], m2, bk_sb, KK5)
                qk_drain(psq[4 + m2], m2, bq_sb, QQ5)

        nc.sync.dma_start(out=WOP[0], in_=wop[0])
        nc.gpsimd.dma_start(out=WOP[1], in_=wop[1])

        # ---------- main loop ----------
        fillers = []

        with (
            tc.tile_pool(name="psS", bufs=1, space="PSUM") as psS,
            tc.tile_pool(name="psO", bufs=1, space="PSUM") as psO,
            tc.tile_pool(name="psZ", bufs=1, space="PSUM") as psZ,
            tc.tile_pool(name="psC", bufs=1, space="PSUM") as psC,
        ):
            AQ = QQ.rearrange("p (a q) -> p a q", a=2)
            AK = KK.rearrange("p (a q) -> p a q", a=2)

            def emit_projqk(nm, m2, b_sb, d5):
                t = psC.tile([128, RB], F32, tag="c", name=f"f{nm}{m2}")
                for p in range(8):
                    nc.tensor.matmul(
                        t[:], W[(nm, p)][:, 128 * m2:128 * (m2 + 1)],
                        x3o[:, p, :], start=(p == 0), stop=(p == 7))
                qk_drain(t, m2, b_sb, d5)

            def emit_projv(l, cc):
                t = psC.tile([128, 512], F32, tag="c", name=f"fv{l}{cc}")
                for p in range(8):
                    nc.tensor.matmul(
                        t[:], x3o[:, p, 128 * l:128 * (l + 1)],
                        W[("v", p)][:, 512 * cc:512 * (cc + 1)],
                        start=(p == 0), stop=False)
                nc.tensor.matmul(
                    t[:], ones_r[0:1, 0:128],
                    bv_sb[0:1, 512 * cc:512 * (cc + 1)],
                    start=False, stop=True)
                nc.vector.tensor_copy(VN[l][:, 512 * cc:512 * (cc + 1)], t[:])

            # fillers: list of CHUNKS (list of closures). A multi-closure
            # chunk owns the psC bank from first to last closure, so it must
            # never straddle a point where norm_b/outproj also allocate psC
            # (PE-FIFO vs psum-WAR inversion = deadlock). pop_fillers only
            # starts a multi-closure chunk early enough in the unit.
            def make_projqk_fillers(nm, m2, b_sb, d5):
                st = {}
                chunk = []

                def mk(p):
                    def go():
                        if p == 0:
                            st["t"] = psC.tile([128, RB], F32, tag="c",
                                               name=f"f{nm}{m2}")
                        nc.tensor.matmul(
                            st["t"][:],
                            W[(nm, p)][:, 128 * m2:128 * (m2 + 1)],
                            x3o[:, p, :], start=(p == 0), stop=(p == 7))
                        if p == 7:
                            qk_drain(st["t"], m2, b_sb, d5)
                    return go
                for p in range(8):
                    chunk.append(mk(p))
                fillers.append(chunk)

            def make_projv_fillers(l, cc):
                st = {}
                chunk = []

                def mk(p):
                    def go():
                        if p == 0:
                            st["t"] = psC.tile([128, 512], F32, tag="c",
                                               name=f"fv{l}{cc}")
                        nc.tensor.matmul(
                            st["t"][:], x3o[:, p, 128 * l:128 * (l + 1)],
                            W[("v", p)][:, 512 * cc:512 * (cc + 1)],
                            start=(p == 0), stop=False)
                        if p == 7:
                            nc.tensor.matmul(
                                st["t"][:], ones_r[0:1, 0:128],
                                bv_sb[0:1, 512 * cc:512 * (cc + 1)],
                                start=False, stop=True)
                            nc.vector.tensor_copy(
                                VN[l][:, 512 * cc:512 * (cc + 1)], st["t"][:])
                    return go
                for p in range(8):
                    chunk.append(mk(p))
                fillers.append(chunk)

            def make_outproj_full(sqs):
                def mk(sq, cc):
                    def go():
                        oc = psC.tile([128, 512], F32, tag="c",
                                      name=f"oc{sq}{cc}")
                        for a in range(2):
                            nc.tensor.matmul(
                                oc[:], ONP[a][:, 128 * sq:128 * sq + 128],
                                WOP[a][:, 512 * cc:512 * (cc + 1)],
                                start=(a == 0), stop=(a == 1))
                        stgc = stgc_pool.tile([128, 512], BF16, tag="sc",
                                              name=f"sc{sq}{cc}")
                        nc.vector.tensor_copy(stgc[:], oc[:])
                        eng = nc.sync if (sq + cc) % 2 == 0 else nc.gpsimd
                        eng.dma_start(
                            out=out[128 * sq:128 * (sq + 1),
                                    512 * cc:512 * (cc + 1)],
                            in_=stgc[:])
                    return go
                for sq in sqs:
                    for cc in range(2):
                        fillers.append([mk(sq, cc)])

            def make_outproj_a0(sqs):
                def mk(sq, cc):
                    def go():
                        oc = psC.tile([128, 512], F32, tag="c",
                                      name=f"pa{sq}{cc}")
                        nc.tensor.matmul(
                            oc[:], ONP[0][:, 128 * sq:128 * sq + 128],
                            WOP[0][:, 512 * cc:512 * (cc + 1)],
                            start=True, stop=True)
                        nc.vector.tensor_copy(
                            OP0[:, 512 * (2 * (sq - 8) + cc):
                                512 * (2 * (sq - 8) + cc) + 512], oc[:])
                    return go
                for sq in sqs:
                    for cc in range(2):
                        fillers.append([mk(sq, cc)])

            sp_t = {}
            pt_t = {}
            st_o = {}
            st_z = {}

            def emit_S(g, z):
                u, s = divmod(g, 16)
                a, hf = UNITS[u]
                zs = slice(64 * z, 64 * z + 64)
                sp = psS.tile([128, 1024], F32, tag=f"s{z}",
                              name=f"ps{g}_{z}")
                sp_t[(g, z)] = sp
                for qc in range(2):
                    nc.tensor.matmul(
                        sp[:, 512 * qc:512 * (qc + 1)],
                        AK[zs, a, 128 * s:128 * s + 128],
                        AQ[zs, a, 1024 * hf + 512 * qc:
                           1024 * hf + 512 * qc + 512],
                        start=True, stop=True)

            def emit_exp(g, z):
                pt = pt_pool.tile([128, 1024], BF16, tag="pt",
                                  name=f"pt{g}_{z}")
                pt_t[(g, z)] = pt
                nc.scalar.activation(pt[:], sp_t.pop((g, z))[:], EXPF,
                                     scale=0.125)

            def emit_PV(g):
                u, s = divmod(g, 16)
                a, hf = UNITS[u]
                if s == 0:
                    st_o[u] = psO.tile([128, 1024], F32, tag="o",
                                       name=f"po{u}")
                    st_z[u] = psZ.tile([128, 512], F32, tag="z",
                                       name=f"pz{u}")
                op, zp = st_o[u], st_z[u]
                pts = [pt_t.pop((g, 0)), pt_t.pop((g, 1))]
                for qc in range(2):
                    for z in range(2):
                        l = 2 * a + z
                        for c in range(2):
                            nc.tensor.matmul(
                                op[64 * z + 32 * c:64 * z + 32 * c + 32,
                                   512 * qc:512 * (qc + 1)],
                                VN[l][:, 64 * s + 32 * c:64 * s + 32 * c + 32],
                                pts[z][:, 512 * qc:512 * (qc + 1)],
                                start=(s == 0), stop=(s == 15),
                                tile_position=(0, 64 * z + 32 * c))
                for z in range(2):
                    for qc in range(2):
                        r = 64 * z + 32 * qc
                        nc.tensor.matmul(
                            zp[r:r + 1, :], onescol[:, 0:1],
                            pts[z][:, 512 * qc:512 * (qc + 1)],
                            start=(s == 0), stop=(s == 15),
                            tile_position=(0, r))

            def emit_norm_a(u):
                zp = st_z.pop(u)
                with nc.allow_low_precision(
                        reason="softmax denom reciprocal in bf16: ~0.4% "
                               "rel, inside the 2e-2 gate"):
                    nc.vector.reciprocal(recZ[0:97, :], zp[0:97, :])

            def emit_norm_b(u):
                a, hf = UNITS[u]
                op = st_o.pop(u)
                for qc in range(2):
                    bc = psC.tile([128, 512], F32, tag="c", name=f"bc{u}{qc}")
                    for z in range(2):
                        r = 64 * z + 32 * qc
                        nc.tensor.matmul(
                            bc[64 * z:64 * z + 64, :],
                            onesb[r:r + 1, :], recZ[r:r + 1, :],
                            start=True, stop=True,
                            tile_position=(r, 64 * z))
                    bcs = stg_pool.tile([128, 512], BF16, tag="bcs",
                                        name=f"bcs{u}{qc}")
                    nc.vector.tensor_copy(bcs[:], bc[:])
                    nc.vector.tensor_mul(
                        ONP[a][:, 1024 * hf + 512 * qc:
                               1024 * hf + 512 * qc + 512],
                        op[:, 512 * qc:512 * (qc + 1)], bcs[:])

            # filler queue with EMISSION deadlines: a producer chunk must be
            # fully emitted before the cycle whose S/PV emission reads its
            # output, else the consumer silently reads garbage (deps only
            # point backwards in program order).
            make_projv_fillers(0, 1)
            make_projv_fillers(1, 1)
            for m2 in range(6, 8):
                make_projqk_fillers("k", m2, bk_sb, KK5)
            make_projv_fillers(2, 0)
            make_projv_fillers(3, 0)
            make_projv_fillers(2, 1)
            make_projv_fillers(3, 1)
            for m2 in range(5, 8):
                make_projqk_fillers("q", m2, bq_sb, QQ5)
            DUE = [8, 8, 2 * 6 - 2, 2 * 7 - 2, 16, 16, 24, 24,
                   30, 30, 30]
            dues = {id(ch): d for ch, d in zip(fillers, DUE)}

            cur_chunk = []

            def force_due(g):
                while fillers and dues.get(id(fillers[0]), 9999) <= g:
                    while cur_chunk:
                        cur_chunk.pop(0)()
                    cur_chunk.extend(fillers.pop(0))
                    while cur_chunk:
                        cur_chunk.pop(0)()

            def pop_fillers(n, s):
                for _ in range(n):
                    if not cur_chunk:
                        if not fillers:
                            return
                        if len(fillers[0]) > 1 and (s >= 14 or s == 0):
                            return  # don't start a chunk near a boundary
                        cur_chunk.extend(fillers.pop(0))
                    cur_chunk.pop(0)()

            # S(0) first so exps start the moment K0/Q0-3 drain; the late
            # prologue chunks stream behind it in the PE FIFO.
            emit_S(0, 0)
            emit_S(0, 1)
            emit_projv(0, 0)
            emit_projv(1, 0)
            emit_projqk("k", 4, bk_sb, KK5)
            emit_projqk("k", 5, bk_sb, KK5)
            emit_projqk("q", 4, bq_sb, QQ5)

            for g in range(NG):
                u, s = divmod(g, 16)
                force_due(g)
                if g == 35:
                    make_outproj_full(range(8))
                if g == 50:
                    make_outproj_a0(range(8, 16))
                if g > 0:
                    # norm_b reads the previous unit's psO tile; it must be
                    # emitted BEFORE emit_PV(g-1) reallocates that ring slot
                    # at s==1 (use-after-realloc inverts the psum WAR).
                    if s == 1 and u > 0:
                        emit_norm_b(u - 1)
                    emit_PV(g - 1)
                    if s == 0 and u > 0:
                        emit_norm_a(u - 1)
                emit_exp(g, 0)
                pop_fillers(2 if g < 31 else 1, s)
                if g + 1 < NG:
                    emit_S(g + 1, 0)
                pop_fillers(1, s)
                emit_exp(g, 1)
                if g + 1 < NG:
                    emit_S(g + 1, 1)
            emit_PV(NG - 1)
            emit_norm_a(3)
            emit_norm_b(3)
            while fillers or cur_chunk:
                pop_fillers(1, 5)

        # ---------- tail: sq 8-15 = pair-1 matmul + add of pair-0 partial
        with tc.tile_pool(name="psT", bufs=3, space="PSUM") as psT:
            for sq in range(8, 16):
                for cc in range(2):
                    oc = psT.tile([128, 512], F32, tag="t",
                                  name=f"tl{sq}{cc}")
                    nc.tensor.matmul(
                        oc[:], ONP[1][:, 128 * sq:128 * sq + 128],
                        WOP[1][:, 512 * cc:512 * (cc + 1)],
                        start=True, stop=True)
                    stgc = stgc_pool.tile([128, 512], BF16, tag="sc",
                                          name=f"tc{sq}{cc}")
                    k = 2 * (sq - 8) + cc
                    if (sq + cc) % 2 == 0:
                        nc.vector.tensor_add(
                            stgc[:], oc[:], OP0[:, 512 * k:512 * k + 512])
                    else:
                        nc.vector.tensor_add(
                            stgc[:], oc[:], OP0[:, 512 * k:512 * k + 512])
                    eng = (nc.sync, nc.gpsimd, nc.scalar)[(sq + cc) % 3]
                    eng.dma_start(
                        out=out[128 * sq:128 * (sq + 1),
                                512 * cc:512 * (cc + 1)],
                        in_=stgc[:])


_NC_CACHE = None


def _get_program():
    global _NC_CACHE
    if _NC_CACHE is None:
        _NC_CACHE = build_program()
    return _NC_CACHE


def _prep_host(x, wq, bq, wk, bk, wv, bv, wo, bo, cos, sin):
    f32 = np.float32
    bf = ml_dtypes.bfloat16
    x = np.asarray(x, f32)
    wq, wk, wv, wo = (np.asarray(a, f32) for a in (wq, wk, wv, wo))
    bq, bk, bv, bo = (np.asarray(a, f32) for a in (bq, bk, bv, bo))
    cos, sin = np.asarray(cos, f32), np.asarray(sin, f32)

    c_row = cos[T]
    s_row = sin[T]
    Cv = np.tile(c_row, H)
    Sv = np.tile(s_row, H)
    sgn = np.where(np.arange(DM) % 2 == 0, -1.0, 1.0).astype(f32)
    Ss = (sgn * Sv).astype(f32)
    swap = np.arange(DM) ^ 1

    wq_rot = Cv[:, None] * wq + Ss[:, None] * wq[swap, :]
    wk_rot = Cv[:, None] * wk + Ss[:, None] * wk[swap, :]
    bq_rot = Cv * bq + Ss * bq[swap]
    bk_rot = Cv * bk + Ss * bk[swap]

    wqTc = np.ascontiguousarray(wq_rot.T).reshape(8, 128, DM).astype(bf)
    wkTc = np.ascontiguousarray(wk_rot.T).reshape(8, 128, DM).astype(bf)
    wvTc = np.ascontiguousarray(wv.T).reshape(8, 128, DM).astype(bf)
    bqp = np.ascontiguousarray(bq_rot.reshape(8, 128).T).astype(f32)
    bkp = np.ascontiguousarray(bk_rot.reshape(8, 128).T).astype(f32)

    in_maps = []
    for i in range(N_CORES):
        b, j = i // 4, i % 4
        xT = x[b, RB * j:RB * (j + 1), :].T
        xTp = np.ascontiguousarray(
            xT.reshape(8, 128, RB).transpose(1, 0, 2).reshape(128, 8 * RB)
        ).astype(bf)
        wopc = np.stack([
            np.ascontiguousarray(
                wo[:, 256 * j + 128 * a:256 * j + 128 * (a + 1)].T)
            for a in range(2)
        ]).astype(bf)
        in_maps.append({
            "xTp": xTp, "wqT": wqTc, "wkT": wkTc, "wvT": wvTc, "wop": wopc,
            "bqp": bqp, "bkp": bkp, "bvr": bv.reshape(1, DM),
            "ones1": np.ones((1, 128), f32),
        })
    return in_maps, bo


def kernel(x, wq, bq, wk, bk, wv, bv, wo, bo, cos, sin,
           _trace=False, _trace_kwargs=None):
    nc = _get_program()
    in_maps, bo_np = _prep_host(x, wq, bq, wk, bk, wv, bv, wo, bo, cos, sin)
    kw = {}
    if _trace:
        kw["trace"] = True
        if _trace_kwargs:
            kw.update(_trace_kwargs)
    res = run_bass_kernel_spmd(nc, in_maps, core_ids=list(range(N_CORES)), **kw)
    outf = np.zeros((B, T, DM), np.float32)
    for i in range(N_CORES):
        part = res.results[i]["out"].astype(np.float32)
        # rows arrive as pi = mp*256 + w*128 + tt; t = tt*16 + 2*mp + w
        part = part.reshape(8, 2, 128, DM).transpose(2, 0, 1, 3).reshape(T, DM)
        outf[i // 4] += part
    outf += bo_np[None, None, :]
    kernel.last_results = res
    return outf
